# revision 1
# baseline (speedup 1.0000x reference)
"""Bass/Trainium2 kernel for nn_CircuitModule (sum-product circuit evaluation).

8 NeuronCores, SPMD, 4 launches (one per circuit layer):
  - Product layers (L0, L2): per-NC slice of output pairs; pairs binned by
    (src-group-of-A, src-group-of-B).  ap_gather fetches per-group candidate
    rows; host 0/1 masks + block-diagonal ones matmul collapse the 16
    candidate partitions per group to one clean [8, F] stream.  Stream B is
    collapsed to DRAM first; stream A's chunks are realigned against B via
    an affine DRAM read and multiplied in the same pass.
  - Sum layers (L1, L3): per-NC contiguous segment range (ix_out is sorted);
    edges binned by src group, dst-sorted within group.  Gather+mask+collapse
    gives per-group streams; tensor_tensor_scan (flag*state + value) builds
    running segment partials; a second ap_gather extracts run-end positions;
    masked full-column ones matmul sums the per-group partials per segment.
  - Pipeline: deep tile-pool buffering; gpsimd runs only ap_gather; input
    DMAs on sync, PSUM drains + output DMAs on the activation engine, mask
    multiply + scan on DVE.  Host work is index-only preprocessing.
"""

import sys

sys.path.insert(0, "/opt/trn_rl_repo")

import numpy as np

import concourse.bacc as bacc
import concourse.mybir as mybir
import concourse.tile as tile
from concourse import bass_utils

F32 = mybir.dt.float32
I16 = mybir.dt.int16

# per-launch HW execution times (ns) from the most recent kernel() call
EXEC_NS = []
TRACE_PATHS = []

NB_VARS = 2_000_000
M0 = 4_000_000
M1 = 1_000_000
M2 = 500_000
M3 = 125_000
NCORES = 8

# x-table geometry: interleaved [pos0, neg0, pos1, neg1, ...], units at 4M
CP = 15_626                 # x_pos entries per partition (padded to 128*CP)
CX = 2 * CP                 # x-table entries per partition (31252 <= 32768)

CH = 1024                   # gather/collapse chunk (columns)


def _pad_to(x, n, val=0):
    out = np.full(n, val, dtype=x.dtype)
    out[: len(x)] = x
    return out


def _align16(n):
    return ((n + 15) // 16) * 16


def _wrap16(idx_groups, F):
    """[8, F] per-group free offsets -> ap_gather wrapped [128, F//16] int16
    (index j of group g is read from partition 16g + j%16, free slot j//16)."""
    assert F % 16 == 0
    assert idx_groups.max(initial=0) < 32768
    out = np.zeros((128, F // 16), dtype=np.int16)
    for g in range(8):
        out[16 * g : 16 * g + 16, :] = (
            idx_groups[g].reshape(F // 16, 16).T.astype(np.int16)
        )
    return out


# ------------------------------------------------------------ host prep


def _prod_maxbin(idxA, idxB, Ctab):
    binid = (idxA // Ctab) // 16 * 8 + (idxB // Ctab) // 16
    return int(np.bincount(binid, minlength=64).max())


def _prep_prod(idxA, idxB, Ctab, K):
    """One core's product-layer prep with forced bin size K (16-aligned)."""
    M = len(idxA)
    F = 8 * K
    a = (idxA // Ctab) // 16
    b = (idxB // Ctab) // 16
    binid = a * 8 + b
    order = np.argsort(binid, kind="stable")
    counts = np.bincount(binid, minlength=64)
    assert counts.max() <= K
    starts = np.zeros(64, dtype=np.int64)
    starts[1:] = np.cumsum(counts)[:-1]
    rank = np.empty(M, dtype=np.int64)
    rank[order] = np.arange(M) - starts[binid[order]]
    colA = b * K + rank                  # column in A row-block a
    colB = a * K + rank                  # column in B row-block b
    store_of_m = a * F + colA            # flat position in the [8, F] output
    qA = (idxA % (16 * Ctab)) // Ctab
    qB = (idxB % (16 * Ctab)) // Ctab
    oA = idxA % Ctab
    oB = idxB % Ctab
    idxAg = np.zeros((8, F), dtype=np.int64)
    idxBg = np.zeros((8, F), dtype=np.int64)
    mskA = np.zeros((128, F), dtype=np.float32)
    mskB = np.zeros((128, F), dtype=np.float32)
    idxAg[a, colA] = oA
    idxBg[b, colB] = oB
    mskA[16 * a + qA, colA] = 1.0
    mskB[16 * b + qB, colB] = 1.0
    return {
        "idxA": _wrap16(idxAg, F),
        "idxB": _wrap16(idxBg, F),
        "mskA": mskA,
        "mskB": mskB,
        "store_of_m": store_of_m,
    }


def _sum_maxcnt(idxE, Ctab):
    return int(np.bincount((idxE // Ctab) // 16, minlength=8).max())


def _prep_sum(idxE, segE, seg_lo, S, Ctab, L, SB):
    """One core's sum-layer prep.  idxE: src table indices; segE: sorted dst
    segment ids; this core owns S segments starting at seg_lo, split into
    128 seg-blocks of SB segments.  Each (group, block) run of the
    dst-sorted group stream is padded to exactly L positions, so group g's
    block p occupies stream positions [p*L, (p+1)*L) and re-tables to
    SBUF partition p affinely.  Run-end partials are then extracted with
    per-partition local_scatter (scat idx: u16-pair -> seg slot, -1 pad)."""
    E = len(idxE)
    F = 128 * L
    g = (idxE // Ctab) // 16
    blk = (segE - seg_lo) // SB
    order = np.argsort(g, kind="stable")     # (g, dst)-sorted; blk monotone
    gs = g[order]
    bs = blk[order]
    ss = segE[order]
    key = gs * 128 + bs
    cnt = np.bincount(key, minlength=1024)
    assert cnt.max() <= L
    starts = np.zeros(1024, dtype=np.int64)
    starts[1:] = np.cumsum(cnt)[:-1]
    rank = np.arange(E) - starts[key]
    pos = bs * L + rank                      # position in group stream
    q = (idxE % (16 * Ctab)) // Ctab
    o = idxE % Ctab
    idxEg = np.zeros((8, F), dtype=np.int64)
    mskE = np.zeros((128, F), dtype=np.float32)
    idxEg[gs, pos] = o[order]
    mskE[16 * gs + q[order], pos] = 1.0
    # scan flags: 0 at first edge of each (group, segment) run, else 1
    segg = np.full((8, F), -1, dtype=np.int64)
    segg[gs, pos] = ss
    flags = np.ones((8, F), dtype=np.float32)
    first = np.ones((8, F), dtype=bool)
    first[:, 1:] = segg[:, 1:] != segg[:, :-1]
    flags[first] = 0.0
    # run-end extraction scatter: scat[g, p, 2t(+1)] = 2d(+1) where t is the
    # local position within block p and d the seg slot within the block
    is_last = np.ones((8, F), dtype=bool)
    is_last[:, :-1] = segg[:, :-1] != segg[:, 1:]
    gg, jj = np.nonzero(is_last & (segg >= 0))
    dd = segg[gg, jj] - seg_lo - (jj // L) * SB
    tt = jj % L
    assert dd.min(initial=0) >= 0 and dd.max(initial=0) < SB
    scat = np.full((8, 128, 2 * L), -1, dtype=np.int16)
    scat[gg, jj // L, 2 * tt] = (2 * dd).astype(np.int16)
    scat[gg, jj // L, 2 * tt + 1] = (2 * dd + 1).astype(np.int16)
    return {
        "idxE": _wrap16(idxEg, F),
        "mskE": mskE,
        "flags": flags,
        "scat": scat.reshape(1024, 2 * L),
    }


# ------------------------------------------------------------ kernels


def _gather_chunk(nc, t_tab, Ctab, idx_dram, msk_dram, c0, w, ip, mp, gp):
    """Issue idx DMA, gather, mask DMA, mask multiply for one chunk.
    Returns the masked [128, w] gather tile."""
    t_idx = ip.tile([128, CH // 16], I16, tag="idx")
    t_g = gp.tile([128, CH], F32, tag="gath")
    t_m = mp.tile([128, CH], F32, tag="mask")
    nc.sync.dma_start(out=t_idx[:, : w // 16],
                      in_=idx_dram[:, c0 // 16 : (c0 + w) // 16])
    nc.gpsimd.ap_gather(
        out_ap=t_g[:, :w].rearrange("p (n d) -> p n d", d=1),
        in_ap=t_tab[:].rearrange("p (n d) -> p n d", d=1),
        idxs_ap=t_idx[:, : w // 16],
        channels=128, num_elems=Ctab, d=1, num_idxs=w,
    )
    nc.sync.dma_start(out=t_m[:, :w], in_=msk_dram[:, c0 : c0 + w])
    nc.vector.tensor_tensor(out=t_g[:, :w], in0=t_g[:, :w], in1=t_m[:, :w],
                            op=mybir.AluOpType.mult)
    return t_g


def _collapse(nc, t_bd, t_g, t_s8, w, pp, rows=8):
    """Block-diag ones matmul [128 -> rows], PSUM drained on act engine."""
    for p0 in range(0, w, 512):
        pw = min(512, w - p0)
        t_ps = pp.tile([rows, 512], F32, tag="ps")
        nc.tensor.matmul(out=t_ps[:, :pw], lhsT=t_bd[:],
                         rhs=t_g[:, p0 : p0 + pw], start=True, stop=True)
        nc.scalar.copy(out=t_s8[:, p0 : p0 + pw], in_=t_ps[:, :pw])


def _build_prod_kernel(Ctab, F, K):
    """out[a, b*K+k] = A[a, b*K+k] * B[b, a*K+k].  B stream is collapsed to
    DRAM first; A's chunks realign B via an affine DRAM read and multiply."""
    nc = bacc.Bacc("TRN2")
    tabin = nc.dram_tensor("tab", [128, Ctab], F32, kind="ExternalInput")
    idxA = nc.dram_tensor("idxA", [128, F // 16], I16, kind="ExternalInput")
    idxB = nc.dram_tensor("idxB", [128, F // 16], I16, kind="ExternalInput")
    mskA = nc.dram_tensor("mskA", [128, F], F32, kind="ExternalInput")
    mskB = nc.dram_tensor("mskB", [128, F], F32, kind="ExternalInput")
    bd = nc.dram_tensor("bd", [128, 8], F32, kind="ExternalInput")
    out = nc.dram_tensor("out", [8, F], F32, kind="ExternalOutput")
    b8d = nc.dram_tensor("b8d", [8, F], F32)

    with tile.TileContext(nc) as tc:
        with (
            tc.tile_pool(name="const", bufs=1) as cp_,
            tc.tile_pool(name="tabp", bufs=1) as tabp,
            tc.tile_pool(name="idxp", bufs=4) as ip,
            tc.tile_pool(name="mskp", bufs=4) as mp,
            tc.tile_pool(name="gp", bufs=4) as gp,
            tc.tile_pool(name="s8p", bufs=4) as sp,
            tc.tile_pool(name="bbp", bufs=4) as bbp,
            tc.tile_pool(name="ps", bufs=4, space="PSUM") as pp,
        ):
            t_bd = cp_.tile([128, 8], F32)
            nc.sync.dma_start(out=t_bd[:], in_=bd[:])
            t_tab = tabp.tile([128, Ctab], F32)
            nc.sync.dma_start(out=t_tab[:], in_=tabin[:])

            # phase B: collapse stream B to DRAM
            for c0 in range(0, F, CH):
                w = min(CH, F - c0)
                t_g = _gather_chunk(nc, t_tab, Ctab, idxB, mskB, c0, w,
                                    ip, mp, gp)
                t_s8 = sp.tile([8, CH], F32, tag="s8")
                _collapse(nc, t_bd, t_g, t_s8, w, pp)
                nc.scalar.dma_start(out=b8d[:, c0 : c0 + w], in_=t_s8[:, :w])

            # phase A: collapse, realign B from DRAM, multiply, store
            for c0 in range(0, F, CH):
                w = min(CH, F - c0)
                t_g = _gather_chunk(nc, t_tab, Ctab, idxA, mskA, c0, w,
                                    ip, mp, gp)
                t_s8 = sp.tile([8, CH], F32, tag="s8")
                _collapse(nc, t_bd, t_g, t_s8, w, pp)
                # pieces of [c0, c0+w) within K-blocks b
                t_bb = bbp.tile([8, CH], F32, tag="bb")
                p0 = c0
                while p0 < c0 + w:
                    b = p0 // K
                    pw = min((b + 1) * K, c0 + w) - p0
                    k0 = p0 - b * K
                    nc.sync.dma_start(
                        out=t_bb[:, p0 - c0 : p0 - c0 + pw],
                        in_=b8d[b].rearrange("(a k) -> a k", k=K)[:, k0 : k0 + pw])
                    p0 += pw
                nc.vector.tensor_tensor(out=t_s8[:, :w], in0=t_s8[:, :w],
                                        in1=t_bb[:, :w],
                                        op=mybir.AluOpType.mult)
                nc.scalar.dma_start(out=out[:, c0 : c0 + w], in_=t_s8[:, :w])
    nc.compile()
    return nc


def _build_sum_kernel(Ctab, L, SB):
    nc = bacc.Bacc("TRN2")
    F = 128 * L
    tabin = nc.dram_tensor("tab", [128, Ctab], F32, kind="ExternalInput")
    idxE = nc.dram_tensor("idxE", [128, F // 16], I16, kind="ExternalInput")
    mskE = nc.dram_tensor("mskE", [128, F], F32, kind="ExternalInput")
    flags = nc.dram_tensor("flags", [8, F], F32, kind="ExternalInput")
    scat = nc.dram_tensor("scat", [1024, 2 * L], I16, kind="ExternalInput")
    bd = nc.dram_tensor("bd", [128, 8], F32, kind="ExternalInput")
    out = nc.dram_tensor("out", [128, SB], F32, kind="ExternalOutput")
    e8d = nc.dram_tensor("e8d", [8, F], F32)

    with tile.TileContext(nc) as tc:
        with (
            tc.tile_pool(name="const", bufs=1) as cp_,
            tc.tile_pool(name="idxp", bufs=4) as ip,
            tc.tile_pool(name="mskp", bufs=4) as mp,
            tc.tile_pool(name="gp", bufs=4) as gp,
            tc.tile_pool(name="ps", bufs=4, space="PSUM") as pp,
        ):
            t_bd = cp_.tile([128, 8], F32)
            t_carry = cp_.tile([8, 1], F32)
            nc.sync.dma_start(out=t_bd[:], in_=bd[:])
            nc.vector.memset(t_carry[:], 0.0)

            # stream phase: gather, collapse, running segment scan -> e8d
            with (
                tc.tile_pool(name="tabp", bufs=1) as tabp,
                tc.tile_pool(name="s8p", bufs=3) as sp,
            ):
                t_tab = tabp.tile([128, Ctab], F32)
                nc.sync.dma_start(out=t_tab[:], in_=tabin[:])
                for c0 in range(0, F, CH):
                    w = min(CH, F - c0)
                    t_g = _gather_chunk(nc, t_tab, Ctab, idxE, mskE, c0, w,
                                        ip, mp, gp)
                    t_s8 = sp.tile([8, CH], F32, tag="s8")
                    _collapse(nc, t_bd, t_g, t_s8, w, pp)
                    t_f = sp.tile([8, CH], F32, tag="flag")
                    t_sc = sp.tile([8, CH], F32, tag="scan")
                    nc.scalar.dma_start(out=t_f[:, :w],
                                        in_=flags[:, c0 : c0 + w])
                    nc.vector.tensor_tensor_scan(
                        out=t_sc[:, :w], data0=t_f[:, :w], data1=t_s8[:, :w],
                        initial=t_carry[:, :1],
                        op0=mybir.AluOpType.mult, op1=mybir.AluOpType.add)
                    nc.vector.tensor_copy(out=t_carry[:],
                                          in_=t_sc[:, w - 1 : w])
                    nc.scalar.dma_start(out=e8d[:, c0 : c0 + w],
                                        in_=t_sc[:, :w])

            # run-end extraction: per group, re-table the padded scan stream
            # to [128, L] (partition = seg-block) and local_scatter the
            # run-end u16 pairs into per-group partial tiles; sum over groups
            with (
                tc.tile_pool(name="xp", bufs=2) as xp,
                tc.tile_pool(name="accp", bufs=1) as ap_,
            ):
                t_acc = ap_.tile([128, 8, 2 * SB], I16)
                for g in range(8):
                    t_rt = xp.tile([128, L], F32, tag="rt")
                    nc.sync.dma_start(
                        out=t_rt[:],
                        in_=e8d[g].rearrange("(p l) -> p l", l=L))
                    t_si = xp.tile([128, 2 * L], I16, tag="si")
                    nc.sync.dma_start(out=t_si[:],
                                      in_=scat[128 * g : 128 * (g + 1), :])
                    nc.gpsimd.local_scatter(
                        out_ap=t_acc[:, g, :],
                        data_ap=t_rt[:].bitcast(I16),
                        idxs_ap=t_si[:],
                        channels=128, num_elems=2 * SB, num_idxs=2 * L)
                t_accf = t_acc.bitcast(F32)     # [128, 8, SB]
                for g in range(1, 8):
                    nc.vector.tensor_tensor(
                        out=t_accf[:, 0, :], in0=t_accf[:, 0, :],
                        in1=t_accf[:, g, :], op=mybir.AluOpType.add)
                nc.scalar.dma_start(out=out[:], in_=t_accf[:, 0, :])
    nc.compile()
    return nc


# ------------------------------------------------------------ driver


def _run(nc, in_maps):
    import os

    if os.environ.get("BASS_PROFILE", "0") == "1":
        try:
            import prof_util

            results, ns, tp = prof_util.run_profiled(nc, in_maps, NCORES)
            if ns is not None:
                EXEC_NS.append(ns)
                TRACE_PATHS.append(tp)
            return results
        except ImportError:
            pass
    res = bass_utils.run_bass_kernel_spmd(
        nc, in_maps, list(range(NCORES)), trace=False)
    if res.exec_time_ns is not None:
        EXEC_NS.append(res.exec_time_ns)
    return res.results


def _bd_mat():
    bd = np.zeros((128, 8), dtype=np.float32)
    for g in range(8):
        bd[16 * g : 16 * g + 16, g] = 1.0
    return bd


def _run_prod_layer(idxA_all, idxB_all, Ctab, tab):
    """idxA_all/idxB_all: [NCORES, Mc].  Returns (stored stream, store pos)."""
    bd = _bd_mat()
    K = _align16(max(_prod_maxbin(idxA_all[k], idxB_all[k], Ctab)
                     for k in range(NCORES)))
    F = 8 * K
    preps = [_prep_prod(idxA_all[k], idxB_all[k], Ctab, K) for k in range(NCORES)]
    nc = _build_prod_kernel(Ctab, F, K)
    in_maps = [
        {"tab": tab, "idxA": p["idxA"], "idxB": p["idxB"], "mskA": p["mskA"],
         "mskB": p["mskB"], "bd": bd}
        for p in preps
    ]
    res = _run(nc, in_maps)
    stream = np.concatenate([res[k]["out"].reshape(-1) for k in range(NCORES)])
    Mc = idxA_all.shape[1]
    pos = np.empty(NCORES * Mc, dtype=np.int64)
    for k in range(NCORES):
        pos[k * Mc : (k + 1) * Mc] = k * 8 * F + preps[k]["store_of_m"]
    return stream, pos


def _run_sum_layer(idxE, segE, nseg, Ctab, tab):
    """Returns the [nseg] segment-sum vector."""
    bd = _bd_mat()
    assert nseg % NCORES == 0
    S = nseg // NCORES
    SB = -(-S // 128)                  # segs per seg-block (partition)
    assert 2 * SB < 2048
    seg_splits = [S * k for k in range(NCORES + 1)]
    edge_splits = np.searchsorted(segE, seg_splits)
    L = 0
    for k in range(NCORES):
        e0, e1 = edge_splits[k], edge_splits[k + 1]
        g = (idxE[e0:e1] // Ctab) // 16
        blk = (segE[e0:e1] - seg_splits[k]) // SB
        L = max(L, int(np.bincount(g * 128 + blk, minlength=1024).max()))
    L = _align16(L)
    preps = []
    for k in range(NCORES):
        e0, e1 = edge_splits[k], edge_splits[k + 1]
        preps.append(_prep_sum(idxE[e0:e1], segE[e0:e1], seg_splits[k], S,
                               Ctab, L, SB))
    nc = _build_sum_kernel(Ctab, L, SB)
    in_maps = [
        {"tab": tab, "idxE": p["idxE"], "mskE": p["mskE"], "flags": p["flags"],
         "scat": p["scat"], "bd": bd}
        for p in preps
    ]
    res = _run(nc, in_maps)
    out = np.empty(nseg, dtype=np.float32)
    for k in range(NCORES):
        out[seg_splits[k] : seg_splits[k + 1]] = res[k]["out"].reshape(-1)[:S]
    return out


def kernel(x_pos, ix_in0, ix_in1, ix_out1, ix_in2, ix_in3, ix_out3):
    x_pos = np.asarray(x_pos, dtype=np.float32)
    ix_in0 = np.asarray(ix_in0, dtype=np.int64)
    ix_in1 = np.asarray(ix_in1, dtype=np.int64)
    ix_out1 = np.asarray(ix_out1, dtype=np.int64)
    ix_in2 = np.asarray(ix_in2, dtype=np.int64)
    ix_in3 = np.asarray(ix_in3, dtype=np.int64)
    ix_out3 = np.asarray(ix_out3, dtype=np.int64)
    EXEC_NS.clear()
    TRACE_PATHS.clear()

    # layer 0: remap units behind the interleaved vars, gather+multiply.
    # x-table is built host-side: [pos0, neg0, pos1, neg1, ..., 0, 1, pad...]
    ix0 = np.where(ix_in0 >= 2, ix_in0 - 2, 2 * NB_VARS + ix_in0)
    xtab = np.zeros(128 * CX, dtype=np.float32)
    xtab[0 : 2 * NB_VARS : 2] = x_pos
    xtab[1 : 2 * NB_VARS : 2] = 1.0 - x_pos
    xtab[2 * NB_VARS] = 0.0
    xtab[2 * NB_VARS + 1] = 1.0
    h0s, pos0 = _run_prod_layer(
        ix0[0::2].reshape(NCORES, -1), ix0[1::2].reshape(NCORES, -1),
        CX, xtab.reshape(128, CX))

    # layer 1: segment sums over h0 stream
    C1 = _align16(-(-len(h0s) // 128))
    assert C1 <= 32768, f"h0 stream table too wide: {C1}"
    tab1 = _pad_to(h0s, 128 * C1).reshape(128, C1)
    h1 = _run_sum_layer(pos0[ix_in1], ix_out1, M1, C1, tab1)

    # layer 2: products over h1 (stored unpermuted)
    C2 = _align16(-(-M1 // 128))
    tab2 = _pad_to(h1, 128 * C2).reshape(128, C2)
    h2s, pos2 = _run_prod_layer(
        ix_in2[0::2].reshape(NCORES, -1), ix_in2[1::2].reshape(NCORES, -1),
        C2, tab=tab2)

    # layer 3: segment sums over h2 stream
    C3 = _align16(-(-len(h2s) // 128))
    tab3 = _pad_to(h2s, 128 * C3).reshape(128, C3)
    h3 = _run_sum_layer(pos2[ix_in3], ix_out3, M3, C3, tab3)
    return h3



# revision 5
# speedup vs baseline: 1.0264x; 1.0264x over previous
"""Bass/Trainium2 kernel for nn_CircuitModule (sum-product circuit evaluation).

8 NeuronCores, SPMD, 4 launches (one per circuit layer):
  - Product layers (L0, L2): per-NC slice of output pairs; pairs binned by
    (src-group-of-A, src-group-of-B).  ap_gather fetches per-group candidate
    rows; fp8 0/1 masks + block-diagonal ones matmul (f32r) collapse the 16
    candidate partitions per group to one clean [8, F] stream.  Stream B is
    collapsed to DRAM first; stream A's chunks are realigned against B via
    an affine DRAM read and multiplied with a 2-chunk skew.
  - Sum layers (L1, L3): per-NC contiguous segment range (ix_out is sorted);
    edges binned by src group, dst-sorted within group.  Gather+mask+collapse
    gives per-group streams; tensor_tensor_scan reads the PSUM collapse
    output directly (flag*state + value) building running segment partials
    with a 1-chunk skew; a second pass extracts run-end positions via
    local_scatter; masked adds sum the per-group partials per segment.
  - Engine assignment is feed-forward per chunk (no round trips):
    gpsimd: ap_gather only.  DVE: mask multiply + scan / realign multiply.
    PE: f32r collapse matmuls.  ACT: PSUM drains (prod) + output DMA issue.
    sync: input DMAs.  Host work is index-only preprocessing.
"""

import sys

sys.path.insert(0, "/opt/trn_rl_repo")

import numpy as np

import concourse.bacc as bacc
import concourse.mybir as mybir
import concourse.tile as tile
from concourse import bass_utils

F32 = mybir.dt.float32
F32R = mybir.dt.float32r
BF16 = mybir.dt.bfloat16
F8 = mybir.dt.float8e4
I16 = mybir.dt.int16

MASK_DT = F8
FLAG_DT = BF16

# per-launch HW execution times (ns) from the most recent kernel() call
EXEC_NS = []
TRACE_PATHS = []

NB_VARS = 2_000_000
M0 = 4_000_000
M1 = 1_000_000
M2 = 500_000
M3 = 125_000
NCORES = 8

# x-table geometry: interleaved [pos0, neg0, pos1, neg1, ...], units at 4M
CP = 15_626                 # x_pos entries per partition (padded to 128*CP)
CX = 2 * CP                 # x-table entries per partition (31252 <= 32768)

CH = 1024                   # gather/collapse chunk (columns)


def _pad_to(x, n, val=0):
    out = np.full(n, val, dtype=x.dtype)
    out[: len(x)] = x
    return out


def _align16(n):
    return ((n + 15) // 16) * 16


def _wrap16(idx_groups, F):
    """[8, F] per-group free offsets -> ap_gather wrapped [128, F//16] int16
    (index j of group g is read from partition 16g + j%16, free slot j//16)."""
    assert F % 16 == 0
    assert idx_groups.max(initial=0) < 32768
    out = np.zeros((128, F // 16), dtype=np.int16)
    for g in range(8):
        out[16 * g : 16 * g + 16, :] = (
            idx_groups[g].reshape(F // 16, 16).T.astype(np.int16)
        )
    return out


# ------------------------------------------------------------ host prep


def _prod_maxbin(idxA, idxB, Ctab):
    binid = (idxA // Ctab) // 16 * 8 + (idxB // Ctab) // 16
    return int(np.bincount(binid, minlength=64).max())


def _prep_prod(idxA, idxB, Ctab, K):
    """One core's product-layer prep with forced bin size K (16-aligned)."""
    M = len(idxA)
    F = 8 * K
    a = (idxA // Ctab) // 16
    b = (idxB // Ctab) // 16
    binid = a * 8 + b
    order = np.argsort(binid, kind="stable")
    counts = np.bincount(binid, minlength=64)
    assert counts.max() <= K
    starts = np.zeros(64, dtype=np.int64)
    starts[1:] = np.cumsum(counts)[:-1]
    rank = np.empty(M, dtype=np.int64)
    rank[order] = np.arange(M) - starts[binid[order]]
    colA = b * K + rank                  # column in A row-block a
    colB = a * K + rank                  # column in B row-block b
    store_of_m = a * F + colA            # flat position in the [8, F] output
    qA = (idxA % (16 * Ctab)) // Ctab
    qB = (idxB % (16 * Ctab)) // Ctab
    oA = idxA % Ctab
    oB = idxB % Ctab
    idxAg = np.zeros((8, F), dtype=np.int64)
    idxBg = np.zeros((8, F), dtype=np.int64)
    mskA = np.zeros((128, F), dtype=np.float32)
    mskB = np.zeros((128, F), dtype=np.float32)
    idxAg[a, colA] = oA
    idxBg[b, colB] = oB
    mskA[16 * a + qA, colA] = 1.0
    mskB[16 * b + qB, colB] = 1.0
    mdt = mybir.dt.np(MASK_DT)
    return {
        "idxA": _wrap16(idxAg, F),
        "idxB": _wrap16(idxBg, F),
        "mskA": mskA.astype(mdt),
        "mskB": mskB.astype(mdt),
        "store_of_m": store_of_m,
    }


def _sum_maxcnt(idxE, Ctab):
    return int(np.bincount((idxE // Ctab) // 16, minlength=8).max())


def _prep_sum(idxE, segE, seg_lo, S, Ctab, L, SB):
    """One core's sum-layer prep.  idxE: src table indices; segE: sorted dst
    segment ids; this core owns S segments starting at seg_lo, split into
    128 seg-blocks of SB segments.  Each (group, block) run of the
    dst-sorted group stream is padded to exactly L positions, so group g's
    block p occupies stream positions [p*L, (p+1)*L) and re-tables to
    SBUF partition p affinely.  Run-end partials are then extracted with
    per-partition local_scatter (scat idx: u16-pair -> seg slot, -1 pad)."""
    E = len(idxE)
    F = 128 * L
    g = (idxE // Ctab) // 16
    blk = (segE - seg_lo) // SB
    order = np.argsort(g, kind="stable")     # (g, dst)-sorted; blk monotone
    gs = g[order]
    bs = blk[order]
    ss = segE[order]
    key = gs * 128 + bs
    cnt = np.bincount(key, minlength=1024)
    assert cnt.max() <= L
    starts = np.zeros(1024, dtype=np.int64)
    starts[1:] = np.cumsum(cnt)[:-1]
    rank = np.arange(E) - starts[key]
    pos = bs * L + rank                      # position in group stream
    q = (idxE % (16 * Ctab)) // Ctab
    o = idxE % Ctab
    idxEg = np.zeros((8, F), dtype=np.int64)
    mskE = np.zeros((128, F), dtype=np.float32)
    idxEg[gs, pos] = o[order]
    mskE[16 * gs + q[order], pos] = 1.0
    # scan flags: 0 at first edge of each (group, segment) run, else 1
    segg = np.full((8, F), -1, dtype=np.int64)
    segg[gs, pos] = ss
    flags = np.ones((8, F), dtype=np.float32)
    first = np.ones((8, F), dtype=bool)
    first[:, 1:] = segg[:, 1:] != segg[:, :-1]
    flags[first] = 0.0
    # run-end extraction scatter: scat[g, p, 2t(+1)] = 2d(+1) where t is the
    # local position within block p and d the seg slot within the block
    is_last = np.ones((8, F), dtype=bool)
    is_last[:, :-1] = segg[:, :-1] != segg[:, 1:]
    gg, jj = np.nonzero(is_last & (segg >= 0))
    dd = segg[gg, jj] - seg_lo - (jj // L) * SB
    tt = jj % L
    assert dd.min(initial=0) >= 0 and dd.max(initial=0) < SB
    scat = np.full((8, 128, 2 * L), -1, dtype=np.int16)
    scat[gg, jj // L, 2 * tt] = (2 * dd).astype(np.int16)
    scat[gg, jj // L, 2 * tt + 1] = (2 * dd + 1).astype(np.int16)
    return {
        "idxE": _wrap16(idxEg, F),
        "mskE": mskE.astype(mybir.dt.np(MASK_DT)),
        "flags": flags.astype(mybir.dt.np(FLAG_DT)),
        "scat": scat.reshape(1024, 2 * L),
    }


# ------------------------------------------------------------ kernels


def _gather_chunk(nc, t_tab, Ctab, idx_dram, msk_dram, c0, w, ip, mp, gp):
    """Issue idx DMA, gather, mask DMA, mask multiply for one chunk.
    Returns the masked [128, w] gather tile."""
    t_idx = ip.tile([128, CH // 16], I16, tag="idx")
    t_g = gp.tile([128, CH], F32, tag="gath")
    t_m = mp.tile([128, CH], MASK_DT, tag="mask")
    nc.sync.dma_start(out=t_idx[:, : w // 16],
                      in_=idx_dram[:, c0 // 16 : (c0 + w) // 16])
    nc.gpsimd.ap_gather(
        out_ap=t_g[:, :w].rearrange("p (n d) -> p n d", d=1),
        in_ap=t_tab[:].rearrange("p (n d) -> p n d", d=1),
        idxs_ap=t_idx[:, : w // 16],
        channels=128, num_elems=Ctab, d=1, num_idxs=w,
    )
    nc.sync.dma_start(out=t_m[:, :w], in_=msk_dram[:, c0 : c0 + w])
    nc.vector.tensor_tensor(out=t_g[:, :w], in0=t_g[:, :w], in1=t_m[:, :w],
                            op=mybir.AluOpType.mult)
    return t_g


def _collapse(nc, t_bd, t_g, t_s8, w, pp):
    """Block-diag ones matmul [128 -> 8], PSUM drained on act engine."""
    for p0 in range(0, w, 512):
        pw = min(512, w - p0)
        t_ps = pp.tile([8, 512], F32, tag="ps")
        nc.tensor.matmul(out=t_ps[:, :pw], lhsT=t_bd[:],
                         rhs=t_g[:, p0 : p0 + pw], start=True, stop=True)
        nc.scalar.copy(out=t_s8[:, p0 : p0 + pw], in_=t_ps[:, :pw])


def _build_prod_kernel(Ctab, F, K):
    """out[a, b*K+k] = A[a, b*K+k] * B[b, a*K+k].  B stream is collapsed to
    DRAM first; A's chunks realign B via an affine DRAM read and multiply
    with a 2-chunk skew (DVE never waits on its own downstream)."""
    nc = bacc.Bacc("TRN2")
    tabin = nc.dram_tensor("tab", [128, Ctab], F32, kind="ExternalInput")
    idxA = nc.dram_tensor("idxA", [128, F // 16], I16, kind="ExternalInput")
    idxB = nc.dram_tensor("idxB", [128, F // 16], I16, kind="ExternalInput")
    mskA = nc.dram_tensor("mskA", [128, F], MASK_DT, kind="ExternalInput")
    mskB = nc.dram_tensor("mskB", [128, F], MASK_DT, kind="ExternalInput")
    bd = nc.dram_tensor("bd", [128, 8], F32, kind="ExternalInput")
    out = nc.dram_tensor("out", [8, F], F32, kind="ExternalOutput")
    b8d = nc.dram_tensor("b8d", [8, F], F32)

    with tile.TileContext(nc) as tc:
        with (
            tc.tile_pool(name="const", bufs=1) as cp_,
            tc.tile_pool(name="tabp", bufs=1) as tabp,
            tc.tile_pool(name="idxp", bufs=4) as ip,
            tc.tile_pool(name="mskp", bufs=4) as mp,
            tc.tile_pool(name="gp", bufs=4) as gp,
            tc.tile_pool(name="s8p", bufs=6) as sp,
            tc.tile_pool(name="bbp", bufs=6) as bbp,
            tc.tile_pool(name="ps", bufs=8, space="PSUM") as pp,
        ):
            t_bd = cp_.tile([128, 8], F32)
            nc.sync.dma_start(out=t_bd[:], in_=bd[:])
            t_tab = tabp.tile([128, Ctab], F32)
            nc.sync.dma_start(out=t_tab[:], in_=tabin[:])

            # phase B: collapse stream B to DRAM
            for c0 in range(0, F, CH):
                w = min(CH, F - c0)
                t_g = _gather_chunk(nc, t_tab, Ctab, idxB, mskB, c0, w,
                                    ip, mp, gp)
                t_s8 = sp.tile([8, CH], F32, tag="s8")
                _collapse(nc, t_bd, t_g, t_s8, w, pp)
                nc.scalar.dma_start(out=b8d[:, c0 : c0 + w], in_=t_s8[:, :w])

            # phase A: collapse, realign B from DRAM, multiply (skewed), store
            pend = []

            def flush_one():
                t_s8o, t_bbo, c0o, wo = pend.pop(0)
                nc.vector.tensor_tensor(out=t_s8o[:, :wo], in0=t_s8o[:, :wo],
                                        in1=t_bbo[:, :wo],
                                        op=mybir.AluOpType.mult)
                nc.scalar.dma_start(out=out[:, c0o : c0o + wo],
                                    in_=t_s8o[:, :wo])

            for c0 in range(0, F, CH):
                w = min(CH, F - c0)
                t_g = _gather_chunk(nc, t_tab, Ctab, idxA, mskA, c0, w,
                                    ip, mp, gp)
                t_s8 = sp.tile([8, CH], F32, tag="s8")
                _collapse(nc, t_bd, t_g, t_s8, w, pp)
                # pieces of [c0, c0+w) within K-blocks b
                t_bb = bbp.tile([8, CH], F32, tag="bb")
                p0 = c0
                while p0 < c0 + w:
                    b = p0 // K
                    pw = min((b + 1) * K, c0 + w) - p0
                    k0 = p0 - b * K
                    nc.sync.dma_start(
                        out=t_bb[:, p0 - c0 : p0 - c0 + pw],
                        in_=b8d[b].rearrange("(a k) -> a k", k=K)[:, k0 : k0 + pw])
                    p0 += pw
                pend.append((t_s8, t_bb, c0, w))
                if len(pend) > 2:
                    flush_one()
            while pend:
                flush_one()
    nc.compile()
    return nc


def _build_sum_kernel(Ctab, L, SB):
    nc = bacc.Bacc("TRN2")
    F = 128 * L
    tabin = nc.dram_tensor("tab", [128, Ctab], F32, kind="ExternalInput")
    idxE = nc.dram_tensor("idxE", [128, F // 16], I16, kind="ExternalInput")
    mskE = nc.dram_tensor("mskE", [128, F], MASK_DT, kind="ExternalInput")
    flags = nc.dram_tensor("flags", [8, F], FLAG_DT, kind="ExternalInput")
    scat = nc.dram_tensor("scat", [1024, 2 * L], I16, kind="ExternalInput")
    bd = nc.dram_tensor("bd", [128, 8], F32, kind="ExternalInput")
    out = nc.dram_tensor("out", [128, SB], F32, kind="ExternalOutput")
    e8d = nc.dram_tensor("e8d", [8, F], F32)

    with tile.TileContext(nc) as tc:
        with (
            tc.tile_pool(name="const", bufs=1) as cp_,
            tc.tile_pool(name="idxp", bufs=4) as ip,
            tc.tile_pool(name="mskp", bufs=4) as mp,
            tc.tile_pool(name="gp", bufs=4) as gp,
            tc.tile_pool(name="ps", bufs=8, space="PSUM") as pp,
        ):
            t_bd = cp_.tile([128, 8], F32)
            nc.sync.dma_start(out=t_bd[:], in_=bd[:])

            # stream phase: gather, collapse, running segment scan -> e8d.
            # The scan reads the PSUM collapse output directly; scans for
            # chunk k are emitted after the mask multiply of chunk k+1 so
            # DVE never stalls on the PE matmuls of its own chunk.
            with (
                tc.tile_pool(name="tabp", bufs=1) as tabp,
                tc.tile_pool(name="fp", bufs=4) as fp_,
                tc.tile_pool(name="scp", bufs=4) as scp,
            ):
                t_tab = tabp.tile([128, Ctab], F32)
                nc.sync.dma_start(out=t_tab[:], in_=tabin[:])
                carry = [0.0]
                pend = []

                def flush_one():
                    pieces, t_fo, t_sco, c0o, wo = pend.pop(0)
                    for p0, pw, t_ps in pieces:
                        nc.vector.tensor_tensor_scan(
                            out=t_sco[:, p0 : p0 + pw],
                            data0=t_fo[:, p0 : p0 + pw],
                            data1=t_ps[:, :pw],
                            initial=carry[0],
                            op0=mybir.AluOpType.mult,
                            op1=mybir.AluOpType.add)
                        carry[0] = t_sco[:, p0 + pw - 1 : p0 + pw]
                    nc.scalar.dma_start(out=e8d[:, c0o : c0o + wo],
                                        in_=t_sco[:, :wo])

                for c0 in range(0, F, CH):
                    w = min(CH, F - c0)
                    t_g = _gather_chunk(nc, t_tab, Ctab, idxE, mskE, c0, w,
                                        ip, mp, gp)
                    t_f = fp_.tile([8, CH], FLAG_DT, tag="flag")
                    nc.sync.dma_start(out=t_f[:, :w],
                                      in_=flags[:, c0 : c0 + w])
                    t_sc = scp.tile([8, CH], F32, tag="scan")
                    pieces = []
                    for p0 in range(0, w, 512):
                        pw = min(512, w - p0)
                        t_ps = pp.tile([8, 512], F32, tag="ps")
                        nc.tensor.matmul(out=t_ps[:, :pw],
                                         lhsT=t_bd[:],
                                         rhs=t_g[:, p0 : p0 + pw],
                                         start=True, stop=True)
                        pieces.append((p0, pw, t_ps))
                    pend.append((pieces, t_f, t_sc, c0, w))
                    if len(pend) > 1:
                        flush_one()
                while pend:
                    flush_one()

            # run-end extraction: per group, re-table the padded scan stream
            # to [128, L] (partition = seg-block) and local_scatter the
            # run-end u16 pairs into per-group partial tiles; sum over groups
            with (
                tc.tile_pool(name="xp", bufs=2) as xp,
                tc.tile_pool(name="accp", bufs=1) as ap_,
            ):
                t_acc = ap_.tile([128, 8, 2 * SB], I16)
                for g in range(8):
                    t_rt = xp.tile([128, L], F32, tag="rt")
                    nc.sync.dma_start(
                        out=t_rt[:],
                        in_=e8d[g].rearrange("(p l) -> p l", l=L))
                    t_si = xp.tile([128, 2 * L], I16, tag="si")
                    nc.sync.dma_start(out=t_si[:],
                                      in_=scat[128 * g : 128 * (g + 1), :])
                    nc.gpsimd.local_scatter(
                        out_ap=t_acc[:, g, :],
                        data_ap=t_rt[:].bitcast(I16),
                        idxs_ap=t_si[:],
                        channels=128, num_elems=2 * SB, num_idxs=2 * L)
                t_accf = t_acc.bitcast(F32)     # [128, 8, SB]
                for g in range(1, 8):
                    nc.vector.tensor_tensor(
                        out=t_accf[:, 0, :], in0=t_accf[:, 0, :],
                        in1=t_accf[:, g, :], op=mybir.AluOpType.add)
                nc.scalar.dma_start(out=out[:], in_=t_accf[:, 0, :])
    nc.compile()
    return nc


# ------------------------------------------------------------ driver


def _run(nc, in_maps):
    import os

    if os.environ.get("BASS_PROFILE", "0") == "1":
        try:
            import prof_util

            results, ns, tp = prof_util.run_profiled(nc, in_maps, NCORES)
            if ns is not None:
                EXEC_NS.append(ns)
                TRACE_PATHS.append(tp)
            return results
        except ImportError:
            pass
    res = bass_utils.run_bass_kernel_spmd(
        nc, in_maps, list(range(NCORES)), trace=False)
    if res.exec_time_ns is not None:
        EXEC_NS.append(res.exec_time_ns)
    return res.results


def _bd_mat():
    bd = np.zeros((128, 8), dtype=np.float32)
    for g in range(8):
        bd[16 * g : 16 * g + 16, g] = 1.0
    return bd


def _run_prod_layer(idxA_all, idxB_all, Ctab, tab):
    """idxA_all/idxB_all: [NCORES, Mc].  Returns (stored stream, store pos)."""
    bd = _bd_mat()
    K = _align16(max(_prod_maxbin(idxA_all[k], idxB_all[k], Ctab)
                     for k in range(NCORES)))
    F = 8 * K
    preps = [_prep_prod(idxA_all[k], idxB_all[k], Ctab, K) for k in range(NCORES)]
    nc = _build_prod_kernel(Ctab, F, K)
    in_maps = [
        {"tab": tab, "idxA": p["idxA"], "idxB": p["idxB"], "mskA": p["mskA"],
         "mskB": p["mskB"], "bd": bd}
        for p in preps
    ]
    res = _run(nc, in_maps)
    stream = np.concatenate([res[k]["out"].reshape(-1) for k in range(NCORES)])
    Mc = idxA_all.shape[1]
    pos = np.empty(NCORES * Mc, dtype=np.int64)
    for k in range(NCORES):
        pos[k * Mc : (k + 1) * Mc] = k * 8 * F + preps[k]["store_of_m"]
    return stream, pos


def _run_sum_layer(idxE, segE, nseg, Ctab, tab):
    """Returns the [nseg] segment-sum vector."""
    bd = _bd_mat()
    assert nseg % NCORES == 0
    S = nseg // NCORES
    SB = -(-S // 128)                  # segs per seg-block (partition)
    assert 2 * SB < 2048
    seg_splits = [S * k for k in range(NCORES + 1)]
    edge_splits = np.searchsorted(segE, seg_splits)
    L = 0
    for k in range(NCORES):
        e0, e1 = edge_splits[k], edge_splits[k + 1]
        g = (idxE[e0:e1] // Ctab) // 16
        blk = (segE[e0:e1] - seg_splits[k]) // SB
        L = max(L, int(np.bincount(g * 128 + blk, minlength=1024).max()))
    L = _align16(L)
    preps = []
    for k in range(NCORES):
        e0, e1 = edge_splits[k], edge_splits[k + 1]
        preps.append(_prep_sum(idxE[e0:e1], segE[e0:e1], seg_splits[k], S,
                               Ctab, L, SB))
    nc = _build_sum_kernel(Ctab, L, SB)
    in_maps = [
        {"tab": tab, "idxE": p["idxE"], "mskE": p["mskE"], "flags": p["flags"],
         "scat": p["scat"], "bd": bd}
        for p in preps
    ]
    res = _run(nc, in_maps)
    out = np.empty(nseg, dtype=np.float32)
    for k in range(NCORES):
        out[seg_splits[k] : seg_splits[k + 1]] = res[k]["out"].reshape(-1)[:S]
    return out


def kernel(x_pos, ix_in0, ix_in1, ix_out1, ix_in2, ix_in3, ix_out3):
    x_pos = np.asarray(x_pos, dtype=np.float32)
    ix_in0 = np.asarray(ix_in0, dtype=np.int64)
    ix_in1 = np.asarray(ix_in1, dtype=np.int64)
    ix_out1 = np.asarray(ix_out1, dtype=np.int64)
    ix_in2 = np.asarray(ix_in2, dtype=np.int64)
    ix_in3 = np.asarray(ix_in3, dtype=np.int64)
    ix_out3 = np.asarray(ix_out3, dtype=np.int64)
    EXEC_NS.clear()
    TRACE_PATHS.clear()

    # layer 0: remap units behind the interleaved vars, gather+multiply.
    # x-table is built host-side: [pos0, neg0, pos1, neg1, ..., 0, 1, pad...]
    ix0 = np.where(ix_in0 >= 2, ix_in0 - 2, 2 * NB_VARS + ix_in0)
    xtab = np.zeros(128 * CX, dtype=np.float32)
    xtab[0 : 2 * NB_VARS : 2] = x_pos
    xtab[1 : 2 * NB_VARS : 2] = 1.0 - x_pos
    xtab[2 * NB_VARS] = 0.0
    xtab[2 * NB_VARS + 1] = 1.0
    h0s, pos0 = _run_prod_layer(
        ix0[0::2].reshape(NCORES, -1), ix0[1::2].reshape(NCORES, -1),
        CX, xtab.reshape(128, CX))

    # layer 1: segment sums over h0 stream
    C1 = _align16(-(-len(h0s) // 128))
    assert C1 <= 32768, f"h0 stream table too wide: {C1}"
    tab1 = _pad_to(h0s, 128 * C1).reshape(128, C1)
    h1 = _run_sum_layer(pos0[ix_in1], ix_out1, M1, C1, tab1)

    # layer 2: products over h1 (stored unpermuted)
    C2 = _align16(-(-M1 // 128))
    tab2 = _pad_to(h1, 128 * C2).reshape(128, C2)
    h2s, pos2 = _run_prod_layer(
        ix_in2[0::2].reshape(NCORES, -1), ix_in2[1::2].reshape(NCORES, -1),
        C2, tab=tab2)

    # layer 3: segment sums over h2 stream
    C3 = _align16(-(-len(h2s) // 128))
    tab3 = _pad_to(h2s, 128 * C3).reshape(128, C3)
    h3 = _run_sum_layer(pos2[ix_in3], ix_out3, M3, C3, tab3)
    return h3


# revision 10
# speedup vs baseline: 1.0325x; 1.0059x over previous
"""Bass/Trainium2 kernel for nn_CircuitModule (sum-product circuit evaluation).

8 NeuronCores, SPMD, 4 launches (one per circuit layer):
  - Product layers (L0, L2): per-NC slice of output pairs; pairs binned by
    (src-group-of-A, src-group-of-B).  ap_gather fetches per-group candidate
    rows; fp8 0/1 masks + block-diagonal ones matmul (f32r) collapse the 16
    candidate partitions per group to one clean [8, F] stream.  Stream B is
    collapsed to DRAM first; stream A's chunks are realigned against B via
    an affine DRAM read and multiplied with a 2-chunk skew.
  - Sum layers (L1, L3): per-NC contiguous segment range (ix_out is sorted);
    edges binned by src group, dst-sorted within group.  Gather+mask+collapse
    gives per-group streams; tensor_tensor_scan reads the PSUM collapse
    output directly (flag*state + value) building running segment partials
    with a 1-chunk skew; a second pass extracts run-end positions via
    local_scatter; masked adds sum the per-group partials per segment.
  - Engine assignment is feed-forward per chunk (no round trips):
    gpsimd: ap_gather only.  DVE: mask multiply + scan / realign multiply.
    PE: f32r collapse matmuls.  ACT: PSUM drains (prod) + output DMA issue.
    sync: input DMAs.  Host work is index-only preprocessing.
"""

import sys

sys.path.insert(0, "/opt/trn_rl_repo")

import numpy as np

import concourse.bacc as bacc
import concourse.mybir as mybir
import concourse.tile as tile
from concourse import bass_utils

F32 = mybir.dt.float32
F32R = mybir.dt.float32r
BF16 = mybir.dt.bfloat16
F8 = mybir.dt.float8e4
I16 = mybir.dt.int16

MASK_DT = F8
FLAG_DT = BF16

# per-launch HW execution times (ns) from the most recent kernel() call
EXEC_NS = []
TRACE_PATHS = []

NB_VARS = 2_000_000
M0 = 4_000_000
M1 = 1_000_000
M2 = 500_000
M3 = 125_000
NCORES = 8

# x-table geometry: interleaved [pos0, neg0, pos1, neg1, ...], units at 4M
CP = 15_626                 # x_pos entries per partition (padded to 128*CP)
CX = 2 * CP                 # x-table entries per partition (31252 <= 32768)

CH = 1024                   # gather/collapse chunk (columns)


def _pad_to(x, n, val=0):
    out = np.full(n, val, dtype=x.dtype)
    out[: len(x)] = x
    return out


def _align16(n):
    return ((n + 15) // 16) * 16


def _wrap16(idx_groups, F):
    """[8, F] per-group free offsets -> ap_gather wrapped [128, F//16] int16
    (index j of group g is read from partition 16g + j%16, free slot j//16)."""
    assert F % 16 == 0
    assert idx_groups.max(initial=0) < 32768
    out = np.zeros((128, F // 16), dtype=np.int16)
    for g in range(8):
        out[16 * g : 16 * g + 16, :] = (
            idx_groups[g].reshape(F // 16, 16).T.astype(np.int16)
        )
    return out


# ------------------------------------------------------------ host prep


def _prod_maxbin(idxA, idxB, Ctab):
    binid = (idxA // Ctab) // 16 * 8 + (idxB // Ctab) // 16
    return int(np.bincount(binid, minlength=64).max())


def _prep_prod(idxA, idxB, Ctab, K):
    """One core's product-layer prep with forced bin size K (16-aligned)."""
    M = len(idxA)
    F = 8 * K
    a = (idxA // Ctab) // 16
    b = (idxB // Ctab) // 16
    binid = a * 8 + b
    order = np.argsort(binid, kind="stable")
    counts = np.bincount(binid, minlength=64)
    assert counts.max() <= K
    starts = np.zeros(64, dtype=np.int64)
    starts[1:] = np.cumsum(counts)[:-1]
    rank = np.empty(M, dtype=np.int64)
    rank[order] = np.arange(M) - starts[binid[order]]
    colA = b * K + rank                  # column in A row-block a
    colB = a * K + rank                  # column in B row-block b
    store_of_m = a * F + colA            # flat position in the [8, F] output
    qA = (idxA % (16 * Ctab)) // Ctab
    qB = (idxB % (16 * Ctab)) // Ctab
    oA = idxA % Ctab
    oB = idxB % Ctab
    idxAg = np.zeros((8, F), dtype=np.int64)
    idxBg = np.zeros((8, F), dtype=np.int64)
    mskA = np.zeros((128, F), dtype=np.float32)
    mskB = np.zeros((128, F), dtype=np.float32)
    idxAg[a, colA] = oA
    idxBg[b, colB] = oB
    mskA[16 * a + qA, colA] = 1.0
    mskB[16 * b + qB, colB] = 1.0
    mdt = mybir.dt.np(MASK_DT)
    return {
        "idxA": _wrap16(idxAg, F),
        "idxB": _wrap16(idxBg, F),
        "mskA": mskA.astype(mdt),
        "mskB": mskB.astype(mdt),
        "store_of_m": store_of_m,
    }


def _sum_maxcnt(idxE, Ctab):
    return int(np.bincount((idxE // Ctab) // 16, minlength=8).max())


def _prep_sum(idxE, segE, seg_lo, S, Ctab, L, SB):
    """One core's sum-layer prep.  idxE: src table indices; segE: sorted dst
    segment ids; this core owns S segments starting at seg_lo, split into
    128 seg-blocks of SB segments.  Each (group, block) run of the
    dst-sorted group stream is padded to exactly L positions, so group g's
    block p occupies stream positions [p*L, (p+1)*L) and re-tables to
    SBUF partition p affinely.  Run-end partials are then extracted with
    per-partition local_scatter (scat idx: u16-pair -> seg slot, -1 pad)."""
    E = len(idxE)
    F = 128 * L
    g = (idxE // Ctab) // 16
    blk = (segE - seg_lo) // SB
    order = np.argsort(g, kind="stable")     # (g, dst)-sorted; blk monotone
    gs = g[order]
    bs = blk[order]
    ss = segE[order]
    key = gs * 128 + bs
    cnt = np.bincount(key, minlength=1024)
    assert cnt.max() <= L
    starts = np.zeros(1024, dtype=np.int64)
    starts[1:] = np.cumsum(cnt)[:-1]
    rank = np.arange(E) - starts[key]
    pos = bs * L + rank                      # position in group stream
    q = (idxE % (16 * Ctab)) // Ctab
    o = idxE % Ctab
    idxEg = np.zeros((8, F), dtype=np.int64)
    mskE = np.zeros((128, F), dtype=np.float32)
    idxEg[gs, pos] = o[order]
    mskE[16 * gs + q[order], pos] = 1.0
    # scan flags: 0 at first edge of each (group, segment) run, else 1
    segg = np.full((8, F), -1, dtype=np.int64)
    segg[gs, pos] = ss
    flags = np.ones((8, F), dtype=np.float32)
    first = np.ones((8, F), dtype=bool)
    first[:, 1:] = segg[:, 1:] != segg[:, :-1]
    flags[first] = 0.0
    # run-end extraction scatter: scat[g, p, 2t(+1)] = 2d(+1) where t is the
    # local position within block p and d the seg slot within the block
    is_last = np.ones((8, F), dtype=bool)
    is_last[:, :-1] = segg[:, :-1] != segg[:, 1:]
    gg, jj = np.nonzero(is_last & (segg >= 0))
    dd = segg[gg, jj] - seg_lo - (jj // L) * SB
    tt = jj % L
    assert dd.min(initial=0) >= 0 and dd.max(initial=0) < SB
    scat = np.full((8, 128, 2 * L), -1, dtype=np.int16)
    scat[gg, jj // L, 2 * tt] = (2 * dd).astype(np.int16)
    scat[gg, jj // L, 2 * tt + 1] = (2 * dd + 1).astype(np.int16)
    return {
        "idxE": _wrap16(idxEg, F),
        "mskE": mskE.astype(mybir.dt.np(MASK_DT)),
        "flags": flags.astype(mybir.dt.np(FLAG_DT)),
        "scat": scat.reshape(1024, 2 * L),
    }


# ------------------------------------------------------------ kernels


def _gather_chunk(nc, t_tab, Ctab, idx_dram, msk_dram, c0, w, ip, mp, gp, gbp):
    """Issue idx DMA, gather, mask DMA, mask multiply for one chunk.
    Returns the masked bf16 [128, w] tile (separate from the gather tile so
    the gather's WAR dependency is only the cheap DVE multiply)."""
    t_idx = ip.tile([128, CH // 16], I16, tag="idx")
    t_g = gp.tile([128, CH], F32, tag="gath")
    t_gb = gbp.tile([128, CH], BF16, tag="gathb")
    t_m = mp.tile([128, CH], MASK_DT, tag="mask")
    nc.sync.dma_start(out=t_idx[:, : w // 16],
                      in_=idx_dram[:, c0 // 16 : (c0 + w) // 16])
    nc.gpsimd.ap_gather(
        out_ap=t_g[:, :w].rearrange("p (n d) -> p n d", d=1),
        in_ap=t_tab[:].rearrange("p (n d) -> p n d", d=1),
        idxs_ap=t_idx[:, : w // 16],
        channels=128, num_elems=Ctab, d=1, num_idxs=w,
    )
    nc.sync.dma_start(out=t_m[:, :w], in_=msk_dram[:, c0 : c0 + w])
    nc.vector.tensor_tensor(out=t_gb[:, :w], in0=t_g[:, :w], in1=t_m[:, :w],
                            op=mybir.AluOpType.mult)
    return t_gb


def _collapse(nc, t_bd, t_gb, t_s8, w, pp):
    """Block-diag ones bf16 matmul [128 -> 8], PSUM drained on act engine."""
    for p0 in range(0, w, 512):
        pw = min(512, w - p0)
        t_ps = pp.tile([8, 512], F32, tag="ps")
        nc.tensor.matmul(out=t_ps[:, :pw], lhsT=t_bd[:],
                         rhs=t_gb[:, p0 : p0 + pw], start=True, stop=True)
        nc.scalar.copy(out=t_s8[:, p0 : p0 + pw], in_=t_ps[:, :pw])


def _build_prod_kernel(Ctab, F, K):
    """out[a, b*K+k] = A[a, b*K+k] * B[b, a*K+k].  B stream is collapsed to
    DRAM first; A's chunks realign B via an affine DRAM read and multiply
    with a 2-chunk skew (DVE never waits on its own downstream)."""
    nc = bacc.Bacc("TRN2")
    tabin = nc.dram_tensor("tab", [128, Ctab], F32, kind="ExternalInput")
    idxA = nc.dram_tensor("idxA", [128, F // 16], I16, kind="ExternalInput")
    idxB = nc.dram_tensor("idxB", [128, F // 16], I16, kind="ExternalInput")
    mskA = nc.dram_tensor("mskA", [128, F], MASK_DT, kind="ExternalInput")
    mskB = nc.dram_tensor("mskB", [128, F], MASK_DT, kind="ExternalInput")
    bd = nc.dram_tensor("bd", [128, 8], BF16, kind="ExternalInput")
    out = nc.dram_tensor("out", [8, F], F32, kind="ExternalOutput")
    b8d = nc.dram_tensor("b8d", [8, F], F32)

    with tile.TileContext(nc) as tc:
        with (
            tc.tile_pool(name="const", bufs=1) as cp_,
            tc.tile_pool(name="tabp", bufs=1) as tabp,
            tc.tile_pool(name="idxp", bufs=6) as ip,
            tc.tile_pool(name="mskp", bufs=6) as mp,
            tc.tile_pool(name="gp", bufs=6) as gp,
            tc.tile_pool(name="gbp", bufs=6) as gbp,
            tc.tile_pool(name="s8p", bufs=3) as sp,
            tc.tile_pool(name="bbp", bufs=3) as bbp,
            tc.tile_pool(name="op", bufs=3) as op_,
            tc.tile_pool(name="ps", bufs=8, space="PSUM") as pp,
        ):
            t_bd = cp_.tile([128, 8], BF16)
            nc.sync.dma_start(out=t_bd[:], in_=bd[:])
            t_tab = tabp.tile([128, Ctab], F32)
            nc.sync.dma_start(out=t_tab[:], in_=tabin[:])

            # phase B: collapse stream B to DRAM
            for c0 in range(0, F, CH):
                w = min(CH, F - c0)
                t_gb = _gather_chunk(nc, t_tab, Ctab, idxB, mskB, c0, w,
                                     ip, mp, gp, gbp)
                t_s8 = sp.tile([8, CH], F32, tag="s8")
                _collapse(nc, t_bd, t_gb, t_s8, w, pp)
                nc.scalar.dma_start(out=b8d[:, c0 : c0 + w], in_=t_s8[:, :w])

            # phase A: collapse, realign B from DRAM, multiply (skewed), store
            pend = []

            def flush_one():
                t_s8o, t_bbo, c0o, wo = pend.pop(0)
                t_o = op_.tile([8, CH], F32, tag="ab")
                nc.vector.tensor_tensor(out=t_o[:, :wo], in0=t_s8o[:, :wo],
                                        in1=t_bbo[:, :wo],
                                        op=mybir.AluOpType.mult)
                nc.scalar.dma_start(out=out[:, c0o : c0o + wo],
                                    in_=t_o[:, :wo])

            for c0 in range(0, F, CH):
                w = min(CH, F - c0)
                t_gb = _gather_chunk(nc, t_tab, Ctab, idxA, mskA, c0, w,
                                     ip, mp, gp, gbp)
                t_s8 = sp.tile([8, CH], F32, tag="s8")
                _collapse(nc, t_bd, t_gb, t_s8, w, pp)
                # pieces of [c0, c0+w) within K-blocks b
                t_bb = bbp.tile([8, CH], F32, tag="bb")
                p0 = c0
                while p0 < c0 + w:
                    b = p0 // K
                    pw = min((b + 1) * K, c0 + w) - p0
                    k0 = p0 - b * K
                    nc.sync.dma_start(
                        out=t_bb[:, p0 - c0 : p0 - c0 + pw],
                        in_=b8d[b].rearrange("(a k) -> a k", k=K)[:, k0 : k0 + pw])
                    p0 += pw
                pend.append((t_s8, t_bb, c0, w))
                if len(pend) > 2:
                    flush_one()
            while pend:
                flush_one()
    nc.compile()
    return nc


def _build_sum_kernel(Ctab, L, SB):
    nc = bacc.Bacc("TRN2")
    F = 128 * L
    tabin = nc.dram_tensor("tab", [128, Ctab], F32, kind="ExternalInput")
    idxE = nc.dram_tensor("idxE", [128, F // 16], I16, kind="ExternalInput")
    mskE = nc.dram_tensor("mskE", [128, F], MASK_DT, kind="ExternalInput")
    flags = nc.dram_tensor("flags", [8, F], FLAG_DT, kind="ExternalInput")
    scat = nc.dram_tensor("scat", [1024, 2 * L], I16, kind="ExternalInput")
    bd = nc.dram_tensor("bd", [128, 8], BF16, kind="ExternalInput")
    out = nc.dram_tensor("out", [128, SB], F32, kind="ExternalOutput")
    e8d = nc.dram_tensor("e8d", [8, F], F32)

    with tile.TileContext(nc) as tc:
        with (
            tc.tile_pool(name="const", bufs=1) as cp_,
            tc.tile_pool(name="idxp", bufs=6) as ip,
            tc.tile_pool(name="mskp", bufs=6) as mp,
            tc.tile_pool(name="gp", bufs=6) as gp,
            tc.tile_pool(name="gbp", bufs=6) as gbp,
            tc.tile_pool(name="ps", bufs=8, space="PSUM") as pp,
        ):
            t_bd = cp_.tile([128, 8], BF16)
            nc.sync.dma_start(out=t_bd[:], in_=bd[:])

            # stream phase: gather, collapse, running segment scan -> e8d.
            # The scan reads the PSUM collapse output directly; scans for
            # chunk k are emitted after the mask multiply of chunk k+1 so
            # DVE never stalls on the PE matmuls of its own chunk.
            with (
                tc.tile_pool(name="tabp", bufs=1) as tabp,
                tc.tile_pool(name="fp", bufs=4) as fp_,
                tc.tile_pool(name="scp", bufs=4) as scp,
            ):
                t_tab = tabp.tile([128, Ctab], F32)
                nc.sync.dma_start(out=t_tab[:], in_=tabin[:])
                carry = [0.0]
                pend = []

                def flush_one():
                    pieces, t_fo, t_sco, c0o, wo = pend.pop(0)
                    for p0, pw, t_ps in pieces:
                        nc.vector.tensor_tensor_scan(
                            out=t_sco[:, p0 : p0 + pw],
                            data0=t_fo[:, p0 : p0 + pw],
                            data1=t_ps[:, :pw],
                            initial=carry[0],
                            op0=mybir.AluOpType.mult,
                            op1=mybir.AluOpType.add)
                        carry[0] = t_sco[:, p0 + pw - 1 : p0 + pw]
                    nc.scalar.dma_start(out=e8d[:, c0o : c0o + wo],
                                        in_=t_sco[:, :wo])

                for c0 in range(0, F, CH):
                    w = min(CH, F - c0)
                    t_gb = _gather_chunk(nc, t_tab, Ctab, idxE, mskE, c0, w,
                                         ip, mp, gp, gbp)
                    t_f = fp_.tile([8, CH], FLAG_DT, tag="flag")
                    nc.sync.dma_start(out=t_f[:, :w],
                                      in_=flags[:, c0 : c0 + w])
                    t_sc = scp.tile([8, CH], F32, tag="scan")
                    pieces = []
                    for p0 in range(0, w, 512):
                        pw = min(512, w - p0)
                        t_ps = pp.tile([8, 512], F32, tag="ps")
                        nc.tensor.matmul(out=t_ps[:, :pw],
                                         lhsT=t_bd[:],
                                         rhs=t_gb[:, p0 : p0 + pw],
                                         start=True, stop=True)
                        pieces.append((p0, pw, t_ps))
                    pend.append((pieces, t_f, t_sc, c0, w))
                    if len(pend) > 1:
                        flush_one()
                while pend:
                    flush_one()

            # run-end extraction: per group, re-table the padded scan stream
            # to [128, L] (partition = seg-block) and local_scatter the
            # run-end u16 pairs into per-group partial tiles; sum over groups
            with (
                tc.tile_pool(name="xp", bufs=2) as xp,
                tc.tile_pool(name="accp", bufs=1) as ap_,
            ):
                t_acc = ap_.tile([128, 8, 2 * SB], I16)
                for g in range(8):
                    t_rt = xp.tile([128, L], F32, tag="rt")
                    nc.sync.dma_start(
                        out=t_rt[:],
                        in_=e8d[g].rearrange("(p l) -> p l", l=L))
                    t_si = xp.tile([128, 2 * L], I16, tag="si")
                    nc.sync.dma_start(out=t_si[:],
                                      in_=scat[128 * g : 128 * (g + 1), :])
                    nc.gpsimd.local_scatter(
                        out_ap=t_acc[:, g, :],
                        data_ap=t_rt[:].bitcast(I16),
                        idxs_ap=t_si[:],
                        channels=128, num_elems=2 * SB, num_idxs=2 * L)
                t_accf = t_acc.bitcast(F32)     # [128, 8, SB]
                for g in range(1, 8):
                    nc.vector.tensor_tensor(
                        out=t_accf[:, 0, :], in0=t_accf[:, 0, :],
                        in1=t_accf[:, g, :], op=mybir.AluOpType.add)
                nc.scalar.dma_start(out=out[:], in_=t_accf[:, 0, :])
    nc.compile()
    return nc


# ------------------------------------------------------------ driver


def _run(nc, in_maps):
    import os

    if os.environ.get("BASS_PROFILE", "0") == "1":
        try:
            import prof_util

            results, ns, tp = prof_util.run_profiled(nc, in_maps, NCORES)
            if ns is not None:
                EXEC_NS.append(ns)
                TRACE_PATHS.append(tp)
            return results
        except ImportError:
            pass
    res = bass_utils.run_bass_kernel_spmd(
        nc, in_maps, list(range(NCORES)), trace=False)
    if res.exec_time_ns is not None:
        EXEC_NS.append(res.exec_time_ns)
    return res.results


def _bd_mat():
    bd = np.zeros((128, 8), dtype=np.float32)
    for g in range(8):
        bd[16 * g : 16 * g + 16, g] = 1.0
    return bd.astype(mybir.dt.np(BF16))


def _run_prod_layer(idxA_all, idxB_all, Ctab, tab):
    """idxA_all/idxB_all: [NCORES, Mc].  Returns (stored stream, store pos)."""
    bd = _bd_mat()
    K = _align16(max(_prod_maxbin(idxA_all[k], idxB_all[k], Ctab)
                     for k in range(NCORES)))
    F = 8 * K
    preps = [_prep_prod(idxA_all[k], idxB_all[k], Ctab, K) for k in range(NCORES)]
    nc = _build_prod_kernel(Ctab, F, K)
    in_maps = [
        {"tab": tab, "idxA": p["idxA"], "idxB": p["idxB"], "mskA": p["mskA"],
         "mskB": p["mskB"], "bd": bd}
        for p in preps
    ]
    res = _run(nc, in_maps)
    stream = np.concatenate([res[k]["out"].reshape(-1) for k in range(NCORES)])
    Mc = idxA_all.shape[1]
    pos = np.empty(NCORES * Mc, dtype=np.int64)
    for k in range(NCORES):
        pos[k * Mc : (k + 1) * Mc] = k * 8 * F + preps[k]["store_of_m"]
    return stream, pos


def _run_sum_layer(idxE, segE, nseg, Ctab, tab):
    """Returns the [nseg] segment-sum vector."""
    bd = _bd_mat()
    assert nseg % NCORES == 0
    S = nseg // NCORES
    SB = -(-S // 128)                  # segs per seg-block (partition)
    assert 2 * SB < 2048
    seg_splits = [S * k for k in range(NCORES + 1)]
    edge_splits = np.searchsorted(segE, seg_splits)
    L = 0
    for k in range(NCORES):
        e0, e1 = edge_splits[k], edge_splits[k + 1]
        g = (idxE[e0:e1] // Ctab) // 16
        blk = (segE[e0:e1] - seg_splits[k]) // SB
        L = max(L, int(np.bincount(g * 128 + blk, minlength=1024).max()))
    L = _align16(L)
    preps = []
    for k in range(NCORES):
        e0, e1 = edge_splits[k], edge_splits[k + 1]
        preps.append(_prep_sum(idxE[e0:e1], segE[e0:e1], seg_splits[k], S,
                               Ctab, L, SB))
    nc = _build_sum_kernel(Ctab, L, SB)
    in_maps = [
        {"tab": tab, "idxE": p["idxE"], "mskE": p["mskE"], "flags": p["flags"],
         "scat": p["scat"], "bd": bd}
        for p in preps
    ]
    res = _run(nc, in_maps)
    out = np.empty(nseg, dtype=np.float32)
    for k in range(NCORES):
        out[seg_splits[k] : seg_splits[k + 1]] = res[k]["out"].reshape(-1)[:S]
    return out


def kernel(x_pos, ix_in0, ix_in1, ix_out1, ix_in2, ix_in3, ix_out3):
    x_pos = np.asarray(x_pos, dtype=np.float32)
    ix_in0 = np.asarray(ix_in0, dtype=np.int64)
    ix_in1 = np.asarray(ix_in1, dtype=np.int64)
    ix_out1 = np.asarray(ix_out1, dtype=np.int64)
    ix_in2 = np.asarray(ix_in2, dtype=np.int64)
    ix_in3 = np.asarray(ix_in3, dtype=np.int64)
    ix_out3 = np.asarray(ix_out3, dtype=np.int64)
    EXEC_NS.clear()
    TRACE_PATHS.clear()

    # layer 0: remap units behind the interleaved vars, gather+multiply.
    # x-table is built host-side: [pos0, neg0, pos1, neg1, ..., 0, 1, pad...]
    ix0 = np.where(ix_in0 >= 2, ix_in0 - 2, 2 * NB_VARS + ix_in0)
    xtab = np.zeros(128 * CX, dtype=np.float32)
    xtab[0 : 2 * NB_VARS : 2] = x_pos
    xtab[1 : 2 * NB_VARS : 2] = 1.0 - x_pos
    xtab[2 * NB_VARS] = 0.0
    xtab[2 * NB_VARS + 1] = 1.0
    h0s, pos0 = _run_prod_layer(
        ix0[0::2].reshape(NCORES, -1), ix0[1::2].reshape(NCORES, -1),
        CX, xtab.reshape(128, CX))

    # layer 1: segment sums over h0 stream
    C1 = _align16(-(-len(h0s) // 128))
    assert C1 <= 32768, f"h0 stream table too wide: {C1}"
    tab1 = _pad_to(h0s, 128 * C1).reshape(128, C1)
    h1 = _run_sum_layer(pos0[ix_in1], ix_out1, M1, C1, tab1)

    # layer 2: products over h1 (stored unpermuted)
    C2 = _align16(-(-M1 // 128))
    tab2 = _pad_to(h1, 128 * C2).reshape(128, C2)
    h2s, pos2 = _run_prod_layer(
        ix_in2[0::2].reshape(NCORES, -1), ix_in2[1::2].reshape(NCORES, -1),
        C2, tab=tab2)

    # layer 3: segment sums over h2 stream
    C3 = _align16(-(-len(h2s) // 128))
    tab3 = _pad_to(h2s, 128 * C3).reshape(128, C3)
    h3 = _run_sum_layer(pos2[ix_in3], ix_out3, M3, C3, tab3)
    return h3


# revision 13
# speedup vs baseline: 1.0341x; 1.0016x over previous
"""Bass/Trainium2 kernel for nn_CircuitModule (sum-product circuit evaluation).

8 NeuronCores, SPMD, 4 launches (one per circuit layer).  The environment
duty-throttles each engine to short grants per ~29us cycle, and the Pool
(gpsimd) engine effectively retires one instruction per cycle — so the
design minimizes gpsimd instruction count: few, very wide ap_gathers
(W=6144 columns), and batched local_scatter extraction.

  - Product layers (L0, L2): pairs binned by (src-group-of-A, src-group-of-B)
    with coordinated ranks.  Wide ap_gather + fp8 0/1 mask multiply (DVE,
    bf16 out) + block-diagonal ones bf16 matmul collapse the 16 candidate
    partitions per group; PSUM is drained by ACT into bf16 [8, F] streams
    a8d/b8d in DRAM.  A dense pass 2 re-tiles both streams to 128 partitions
    via affine DMA (partition = (s2, a, sp), strip S = K/2), multiplies on
    DVE and stores the f32 product stream.
  - Sum layers (L1, L3): edges binned by (src group, dst block), dst-sorted;
    wide gather + mask + collapse, ACT-drained to bf16 e8d.  Extraction
    re-tiles each group's stream to [128 blocks, L], scans on DVE
    (flag*state + value, fp32 state), local_scatters run-end values (bf16,
    several groups per call), and sums group partials per segment.
  - Host work is index-only preprocessing.
"""

import sys

sys.path.insert(0, "/opt/trn_rl_repo")

import numpy as np

import concourse.bacc as bacc
import concourse.mybir as mybir
import concourse.tile as tile
from concourse import bass_utils

F32 = mybir.dt.float32
BF16 = mybir.dt.bfloat16
F8 = mybir.dt.float8e4
I16 = mybir.dt.int16

MASK_DT = F8
FLAG_DT = BF16

# per-launch HW execution times (ns) from the most recent kernel() call
EXEC_NS = []
TRACE_PATHS = []

NB_VARS = 2_000_000
M0 = 4_000_000
M1 = 1_000_000
M2 = 500_000
M3 = 125_000
NCORES = 8

# x-table geometry: interleaved [pos0, neg0, pos1, neg1, ...], units at 4M
CP = 15_626                 # x_pos entries per partition (padded to 128*CP)
CX = 2 * CP                 # x-table entries per partition (31252 <= 32768)

W = 6144                    # gather chunk width (columns)
PW = 2048                   # PSUM drain piece width (4 banks)


def _pad_to(x, n, val=0):
    out = np.full(n, val, dtype=x.dtype)
    out[: len(x)] = x
    return out


def _align16(n):
    return ((n + 15) // 16) * 16


def _wrap16(idx_groups, F):
    """[8, F] per-group free offsets -> ap_gather wrapped [128, F//16] int16
    (index j of group g is read from partition 16g + j%16, free slot j//16)."""
    assert F % 16 == 0
    assert idx_groups.max(initial=0) < 32768
    out = np.zeros((128, F // 16), dtype=np.int16)
    for g in range(8):
        out[16 * g : 16 * g + 16, :] = (
            idx_groups[g].reshape(F // 16, 16).T.astype(np.int16)
        )
    return out


# ------------------------------------------------------------ host prep


def _prod_maxbin(idxA, idxB, Ctab):
    binid = (idxA // Ctab) // 16 * 8 + (idxB // Ctab) // 16
    return int(np.bincount(binid, minlength=64).max())


def _prep_prod(idxA, idxB, Ctab, K):
    """One core's product-layer prep with forced bin size K (16-aligned)."""
    M = len(idxA)
    F = 8 * K
    a = (idxA // Ctab) // 16
    b = (idxB // Ctab) // 16
    binid = a * 8 + b
    order = np.argsort(binid, kind="stable")
    counts = np.bincount(binid, minlength=64)
    assert counts.max() <= K
    starts = np.zeros(64, dtype=np.int64)
    starts[1:] = np.cumsum(counts)[:-1]
    rank = np.empty(M, dtype=np.int64)
    rank[order] = np.arange(M) - starts[binid[order]]
    colA = b * K + rank                  # column in A row-block a
    colB = a * K + rank                  # column in B row-block b
    store_of_m = b * F + colB            # flat position in the b-layout output
    qA = (idxA % (16 * Ctab)) // Ctab
    qB = (idxB % (16 * Ctab)) // Ctab
    oA = idxA % Ctab
    oB = idxB % Ctab
    idxAg = np.zeros((8, F), dtype=np.int64)
    idxBg = np.zeros((8, F), dtype=np.int64)
    mskA = np.zeros((128, F), dtype=np.float32)
    mskB = np.zeros((128, F), dtype=np.float32)
    idxAg[a, colA] = oA
    idxBg[b, colB] = oB
    mskA[16 * a + qA, colA] = 1.0
    mskB[16 * b + qB, colB] = 1.0
    mdt = mybir.dt.np(MASK_DT)
    return {
        "idxA": _wrap16(idxAg, F),
        "idxB": _wrap16(idxBg, F),
        "mskA": mskA.astype(mdt),
        "mskB": mskB.astype(mdt),
        "store_of_m": store_of_m,
    }


def _prep_sum(idxE, segE, seg_lo, S, Ctab, L, SB, GP):
    """One core's sum-layer prep.  idxE: src table indices; segE: sorted dst
    segment ids; this core owns S segments starting at seg_lo, split into
    128 seg-blocks of SB segments.  Each (group, block) run of the
    dst-sorted group stream is padded to exactly L positions, so group g's
    block p occupies stream positions [p*L, (p+1)*L) and re-tables to
    SBUF partition p affinely.  Run-end partials (bf16) are extracted with
    local_scatter, GP groups per call (scat idx -> g_local*SB + seg slot)."""
    E = len(idxE)
    F = 128 * L
    g = (idxE // Ctab) // 16
    blk = (segE - seg_lo) // SB
    order = np.argsort(g, kind="stable")     # (g, dst)-sorted; blk monotone
    gs = g[order]
    bs = blk[order]
    ss = segE[order]
    key = gs * 128 + bs
    cnt = np.bincount(key, minlength=1024)
    assert cnt.max() <= L
    starts = np.zeros(1024, dtype=np.int64)
    starts[1:] = np.cumsum(cnt)[:-1]
    rank = np.arange(E) - starts[key]
    pos = bs * L + rank                      # position in group stream
    q = (idxE % (16 * Ctab)) // Ctab
    o = idxE % Ctab
    idxEg = np.zeros((8, F), dtype=np.int64)
    mskE = np.zeros((128, F), dtype=np.float32)
    idxEg[gs, pos] = o[order]
    mskE[16 * gs + q[order], pos] = 1.0
    # scan flags: 0 at first edge of each (group, segment) run, else 1;
    # laid out [128*8, L]: row 128*g + p = flags of group g's block p
    segg = np.full((8, F), -1, dtype=np.int64)
    segg[gs, pos] = ss
    flags = np.ones((8, F), dtype=np.float32)
    first = np.ones((8, F), dtype=bool)
    first[:, 1:] = segg[:, 1:] != segg[:, :-1]
    # block boundaries always start fresh (segments never span blocks)
    first[:, ::L] = True
    flags[first] = 0.0
    # run-end extraction scatter: GP groups per call; for call c, column
    # (g_local*L + t) -> slot g_local*SB + d  (single bf16 value per slot)
    is_last = np.ones((8, F), dtype=bool)
    is_last[:, :-1] = segg[:, :-1] != segg[:, 1:]
    is_last[:, L - 1 :: L] = True
    gg, jj = np.nonzero(is_last & (segg >= 0))
    dd = segg[gg, jj] - seg_lo - (jj // L) * SB
    tt = jj % L
    assert dd.min(initial=0) >= 0 and dd.max(initial=0) < SB
    ncalls = 8 // GP
    scat = np.full((ncalls, 128, GP * L), -1, dtype=np.int16)
    scat[gg // GP, jj // L, (gg % GP) * L + tt] = (
        (gg % GP) * SB + dd
    ).astype(np.int16)
    return {
        "idxE": _wrap16(idxEg, F),
        "mskE": mskE.astype(mybir.dt.np(MASK_DT)),
        "flags": flags.reshape(8, 128, L).reshape(1024, L)
        .astype(mybir.dt.np(FLAG_DT)),
        "scat": scat.reshape(ncalls * 128, GP * L),
    }


# ------------------------------------------------------------ kernels


def _gather_chunk(nc, t_tab, Ctab, idx_dram, msk_dram, c0, w, ip, mp, gp, gbp):
    """Issue idx DMA, wide gather, mask DMA, mask multiply (bf16 out)."""
    t_idx = ip.tile([128, W // 16], I16, tag="idx")
    t_g = gp.tile([128, W], F32, tag="gath")
    t_gb = gbp.tile([128, W], BF16, tag="gathb")
    t_m = mp.tile([128, W], MASK_DT, tag="mask")
    nc.sync.dma_start(out=t_idx[:, : w // 16],
                      in_=idx_dram[:, c0 // 16 : (c0 + w) // 16])
    nc.gpsimd.ap_gather(
        out_ap=t_g[:, :w].rearrange("p (n d) -> p n d", d=1),
        in_ap=t_tab[:].rearrange("p (n d) -> p n d", d=1),
        idxs_ap=t_idx[:, : w // 16],
        channels=128, num_elems=Ctab, d=1, num_idxs=w,
    )
    nc.sync.dma_start(out=t_m[:, :w], in_=msk_dram[:, c0 : c0 + w])
    nc.vector.tensor_tensor(out=t_gb[:, :w], in0=t_g[:, :w], in1=t_m[:, :w],
                            op=mybir.AluOpType.mult)
    return t_gb


def _collapse_store(nc, t_bd, t_gb, w, pp, sp, store):
    """Block-diag ones bf16 matmuls [128 -> 8] in PW pieces; ACT drains each
    piece to a bf16 [8, w] tile and store(tile, piece_lo, piece_w) is called
    to emit the output DMA."""
    for p0 in range(0, w, PW):
        pw = min(PW, w - p0)
        t_ps = pp.tile([8, PW], F32, tag="ps")
        for q0 in range(0, pw, 512):
            qw = min(512, pw - q0)
            nc.tensor.matmul(out=t_ps[:, q0 : q0 + qw], lhsT=t_bd[:],
                             rhs=t_gb[:, p0 + q0 : p0 + q0 + qw],
                             start=True, stop=True)
        t_s8 = sp.tile([8, PW], BF16, tag="s8")
        nc.scalar.copy(out=t_s8[:, :pw], in_=t_ps[:, :pw])
        store(t_s8, p0, pw)


def _build_prod_kernel(Ctab, F, K):
    """Pass 1: collapse streams A and B to bf16 [8, F] DRAM (a8d/b8d).
    Pass 2: out[a, b*K+r] = A[a, b*K+r] * B[b, a*K+r] via affine re-tiling
    to 128 partitions (partition = (s2, a, sp), strip S = K/2)."""
    nc = bacc.Bacc("TRN2")
    S = K // 2
    tabin = nc.dram_tensor("tab", [128, Ctab], F32, kind="ExternalInput")
    idxA = nc.dram_tensor("idxA", [128, F // 16], I16, kind="ExternalInput")
    idxB = nc.dram_tensor("idxB", [128, F // 16], I16, kind="ExternalInput")
    mskA = nc.dram_tensor("mskA", [128, F], MASK_DT, kind="ExternalInput")
    mskB = nc.dram_tensor("mskB", [128, F], MASK_DT, kind="ExternalInput")
    bd = nc.dram_tensor("bd", [128, 8], BF16, kind="ExternalInput")
    out = nc.dram_tensor("out", [8, F], F32, kind="ExternalOutput")
    a8d = nc.dram_tensor("a8d", [8, F], BF16)
    b8d = nc.dram_tensor("b8d", [8, F], BF16)

    with tile.TileContext(nc) as tc:
        with tc.tile_pool(name="const", bufs=1) as cp_:
            t_bd = cp_.tile([128, 8], BF16)
            nc.sync.dma_start(out=t_bd[:], in_=bd[:])
            with (
                tc.tile_pool(name="tabp", bufs=1) as tabp,
                tc.tile_pool(name="idxp", bufs=2) as ip,
                tc.tile_pool(name="mskp", bufs=2) as mp,
                tc.tile_pool(name="gp", bufs=1) as gp,
                tc.tile_pool(name="gbp", bufs=1) as gbp,
                tc.tile_pool(name="s8p", bufs=3) as sp,
                tc.tile_pool(name="ps", bufs=2, space="PSUM") as pp,
            ):
                t_tab = tabp.tile([128, Ctab], F32)
                nc.sync.dma_start(out=t_tab[:], in_=tabin[:])

                def store_b(t_s8, p0, pw, c0):
                    nc.scalar.dma_start(out=b8d[:, c0 + p0 : c0 + p0 + pw],
                                        in_=t_s8[:, :pw])

                def store_a(t_s8, p0, pw, c0):
                    # a8d is in b-layout (flat = b*F + a*K + r); split the
                    # piece's standard col range [y0, y1) at K boundaries
                    y0 = c0 + p0
                    while y0 < c0 + p0 + pw:
                        b = y0 // K
                        yw = min((b + 1) * K, c0 + p0 + pw) - y0
                        r0 = y0 - b * K
                        nc.scalar.dma_start(
                            out=a8d[b].rearrange("(a r) -> a r", r=K)
                            [:, r0 : r0 + yw],
                            in_=t_s8[:, y0 - c0 - p0 : y0 - c0 - p0 + yw])
                        y0 += yw

                for dst, idx_d, msk_d in ((store_b, idxB, mskB),
                                          (store_a, idxA, mskA)):
                    for c0 in range(0, F, W):
                        w = min(W, F - c0)
                        t_gb = _gather_chunk(nc, t_tab, Ctab, idx_d, msk_d,
                                             c0, w, ip, mp, gp, gbp)

                        def store(t_s8, p0, pw, c0=c0, dst=dst):
                            dst(t_s8, p0, pw, c0)

                        _collapse_store(nc, t_bd, t_gb, w, pp, sp, store)

            # pass 2: both streams are now in b-layout (flat = b*F + a*K + r)
            # so the realign multiply is a single contiguous [128, S] pass:
            # partition p = (b, a, sp), x in [0, S).
            av = a8d.rearrange("b (p2 x) -> (b p2) x", p2=16)
            bv = b8d.rearrange("b (p2 x) -> (b p2) x", p2=16)
            ov = out.rearrange("b (p2 x) -> (b p2) x", p2=16)
            with (
                tc.tile_pool(name="pa", bufs=1) as pa,
                tc.tile_pool(name="pb", bufs=1) as pb,
                tc.tile_pool(name="po", bufs=1) as po,
            ):
                t_a = pa.tile([128, S], BF16)
                t_b = pb.tile([128, S], BF16)
                t_o = po.tile([128, S], F32)
                nc.sync.dma_start(out=t_a[:], in_=av[:])
                nc.sync.dma_start(out=t_b[:], in_=bv[:])
                nc.vector.tensor_tensor(out=t_o[:], in0=t_a[:], in1=t_b[:],
                                        op=mybir.AluOpType.mult)
                nc.scalar.dma_start(out=ov[:], in_=t_o[:])
    nc.compile()
    return nc


def _build_sum_kernel(Ctab, L, SB, GP):
    nc = bacc.Bacc("TRN2")
    F = 128 * L
    ncalls = 8 // GP
    tabin = nc.dram_tensor("tab", [128, Ctab], F32, kind="ExternalInput")
    idxE = nc.dram_tensor("idxE", [128, F // 16], I16, kind="ExternalInput")
    mskE = nc.dram_tensor("mskE", [128, F], MASK_DT, kind="ExternalInput")
    flags = nc.dram_tensor("flags", [1024, L], FLAG_DT, kind="ExternalInput")
    scat = nc.dram_tensor("scat", [ncalls * 128, GP * L], I16,
                          kind="ExternalInput")
    bd = nc.dram_tensor("bd", [128, 8], BF16, kind="ExternalInput")
    out = nc.dram_tensor("out", [128, SB], F32, kind="ExternalOutput")
    e8d = nc.dram_tensor("e8d", [8, F], BF16)

    with tile.TileContext(nc) as tc:
        with tc.tile_pool(name="const", bufs=1) as cp_:
            t_bd = cp_.tile([128, 8], BF16)
            nc.sync.dma_start(out=t_bd[:], in_=bd[:])

            # stream phase: wide gather, mask, collapse, drain -> e8d (bf16)
            with (
                tc.tile_pool(name="tabp", bufs=1) as tabp,
                tc.tile_pool(name="idxp", bufs=2) as ip,
                tc.tile_pool(name="mskp", bufs=2) as mp,
                tc.tile_pool(name="gp", bufs=1) as gp,
                tc.tile_pool(name="gbp", bufs=1) as gbp,
                tc.tile_pool(name="s8p", bufs=3) as sp,
                tc.tile_pool(name="ps", bufs=2, space="PSUM") as pp,
            ):
                t_tab = tabp.tile([128, Ctab], F32)
                nc.sync.dma_start(out=t_tab[:], in_=tabin[:])
                for c0 in range(0, F, W):
                    w = min(W, F - c0)
                    t_gb = _gather_chunk(nc, t_tab, Ctab, idxE, mskE, c0, w,
                                         ip, mp, gp, gbp)

                    def store(t_s8, p0, pw, c0=c0):
                        nc.scalar.dma_start(
                            out=e8d[:, c0 + p0 : c0 + p0 + pw],
                            in_=t_s8[:, :pw])

                    _collapse_store(nc, t_bd, t_gb, w, pp, sp, store)

            # extraction: per group re-table [128 blocks, L], scan on DVE,
            # batched local_scatter of run-end bf16 values, sum over groups
            with (
                tc.tile_pool(name="xp", bufs=3) as xp,
                tc.tile_pool(name="scp", bufs=1) as scp,
                tc.tile_pool(name="accp", bufs=1) as ap_,
                tc.tile_pool(name="finp", bufs=1) as fin,
            ):
                t_sc8 = scp.tile([128, 8, L], BF16)
                for g in range(8):
                    t_rt = xp.tile([128, L], BF16, tag="rt")
                    nc.sync.dma_start(
                        out=t_rt[:],
                        in_=e8d[g].rearrange("(p l) -> p l", l=L))
                    t_f = xp.tile([128, L], FLAG_DT, tag="fl")
                    nc.sync.dma_start(out=t_f[:],
                                      in_=flags[128 * g : 128 * (g + 1), :])
                    nc.vector.tensor_tensor_scan(
                        out=t_sc8[:, g, :], data0=t_f[:], data1=t_rt[:],
                        initial=0.0,
                        op0=mybir.AluOpType.mult, op1=mybir.AluOpType.add)
                t_acc = ap_.tile([128, ncalls, GP * SB], I16)
                for c in range(ncalls):
                    t_si = xp.tile([128, GP * L], I16, tag="si")
                    nc.sync.dma_start(
                        out=t_si[:],
                        in_=scat[128 * c : 128 * (c + 1), :])
                    nc.gpsimd.local_scatter(
                        out_ap=t_acc[:, c, :],
                        data_ap=t_sc8[:, GP * c : GP * (c + 1), :]
                        .rearrange("p g l -> p (g l)").bitcast(I16),
                        idxs_ap=t_si[:],
                        channels=128, num_elems=GP * SB, num_idxs=GP * L)
                t_accb = t_acc.bitcast(BF16)    # [128, ncalls, GP, SB]
                t_fin = fin.tile([128, SB], F32)
                acv = t_accb.rearrange("p c (g s) -> p (c g) s", g=GP)
                nc.vector.tensor_copy(out=t_fin[:], in_=acv[:, 0, :])
                for j in range(1, 8):
                    nc.vector.tensor_tensor(
                        out=t_fin[:], in0=t_fin[:], in1=acv[:, j, :],
                        op=mybir.AluOpType.add)
                nc.scalar.dma_start(out=out[:], in_=t_fin[:])
    nc.compile()
    return nc


# ------------------------------------------------------------ driver


def _run(nc, in_maps):
    import os

    if os.environ.get("BASS_PROFILE", "0") == "1":
        try:
            import prof_util

            results, ns, tp = prof_util.run_profiled(nc, in_maps, NCORES)
            if ns is not None:
                EXEC_NS.append(ns)
                TRACE_PATHS.append(tp)
            return results
        except ImportError:
            pass
    res = bass_utils.run_bass_kernel_spmd(
        nc, in_maps, list(range(NCORES)), trace=False)
    if res.exec_time_ns is not None:
        EXEC_NS.append(res.exec_time_ns)
    return res.results


def _bd_mat():
    bd = np.zeros((128, 8), dtype=np.float32)
    for g in range(8):
        bd[16 * g : 16 * g + 16, g] = 1.0
    return bd.astype(mybir.dt.np(BF16))


def _run_prod_layer(idxA_all, idxB_all, Ctab, tab):
    """idxA_all/idxB_all: [NCORES, Mc].  Returns (stored stream, store pos)."""
    bd = _bd_mat()
    K = _align16(max(_prod_maxbin(idxA_all[k], idxB_all[k], Ctab)
                     for k in range(NCORES)))
    if K % 32:
        K += 16                         # keep S = K/2 16-aligned
    F = 8 * K
    preps = [_prep_prod(idxA_all[k], idxB_all[k], Ctab, K) for k in range(NCORES)]
    nc = _build_prod_kernel(Ctab, F, K)
    in_maps = [
        {"tab": tab, "idxA": p["idxA"], "idxB": p["idxB"], "mskA": p["mskA"],
         "mskB": p["mskB"], "bd": bd}
        for p in preps
    ]
    res = _run(nc, in_maps)
    stream = np.concatenate([res[k]["out"].reshape(-1) for k in range(NCORES)])
    Mc = idxA_all.shape[1]
    pos = np.empty(NCORES * Mc, dtype=np.int64)
    for k in range(NCORES):
        pos[k * Mc : (k + 1) * Mc] = k * 8 * F + preps[k]["store_of_m"]
    return stream, pos


def _run_sum_layer(idxE, segE, nseg, Ctab, tab):
    """Returns the [nseg] segment-sum vector."""
    bd = _bd_mat()
    assert nseg % NCORES == 0
    S = nseg // NCORES
    SB = -(-S // 128)                  # segs per seg-block (partition)
    GP = 8
    while GP > 1 and GP * SB > 2046:
        GP //= 2
    assert GP * SB <= 2046
    seg_splits = [S * k for k in range(NCORES + 1)]
    edge_splits = np.searchsorted(segE, seg_splits)
    L = 0
    for k in range(NCORES):
        e0, e1 = edge_splits[k], edge_splits[k + 1]
        g = (idxE[e0:e1] // Ctab) // 16
        blk = (segE[e0:e1] - seg_splits[k]) // SB
        L = max(L, int(np.bincount(g * 128 + blk, minlength=1024).max()))
    L = _align16(L)
    preps = []
    for k in range(NCORES):
        e0, e1 = edge_splits[k], edge_splits[k + 1]
        preps.append(_prep_sum(idxE[e0:e1], segE[e0:e1], seg_splits[k], S,
                               Ctab, L, SB, GP))
    nc = _build_sum_kernel(Ctab, L, SB, GP)
    in_maps = [
        {"tab": tab, "idxE": p["idxE"], "mskE": p["mskE"], "flags": p["flags"],
         "scat": p["scat"], "bd": bd}
        for p in preps
    ]
    res = _run(nc, in_maps)
    out = np.empty(nseg, dtype=np.float32)
    for k in range(NCORES):
        out[seg_splits[k] : seg_splits[k + 1]] = res[k]["out"].reshape(-1)[:S]
    return out


def kernel(x_pos, ix_in0, ix_in1, ix_out1, ix_in2, ix_in3, ix_out3):
    x_pos = np.asarray(x_pos, dtype=np.float32)
    ix_in0 = np.asarray(ix_in0, dtype=np.int64)
    ix_in1 = np.asarray(ix_in1, dtype=np.int64)
    ix_out1 = np.asarray(ix_out1, dtype=np.int64)
    ix_in2 = np.asarray(ix_in2, dtype=np.int64)
    ix_in3 = np.asarray(ix_in3, dtype=np.int64)
    ix_out3 = np.asarray(ix_out3, dtype=np.int64)
    EXEC_NS.clear()
    TRACE_PATHS.clear()

    # layer 0: remap units behind the interleaved vars, gather+multiply.
    # x-table is built host-side: [pos0, neg0, pos1, neg1, ..., 0, 1, pad...]
    ix0 = np.where(ix_in0 >= 2, ix_in0 - 2, 2 * NB_VARS + ix_in0)
    xtab = np.zeros(128 * CX, dtype=np.float32)
    xtab[0 : 2 * NB_VARS : 2] = x_pos
    xtab[1 : 2 * NB_VARS : 2] = 1.0 - x_pos
    xtab[2 * NB_VARS] = 0.0
    xtab[2 * NB_VARS + 1] = 1.0
    h0s, pos0 = _run_prod_layer(
        ix0[0::2].reshape(NCORES, -1), ix0[1::2].reshape(NCORES, -1),
        CX, xtab.reshape(128, CX))

    # layer 1: segment sums over h0 stream
    C1 = _align16(-(-len(h0s) // 128))
    assert C1 <= 32768, f"h0 stream table too wide: {C1}"
    tab1 = _pad_to(h0s, 128 * C1).reshape(128, C1)
    h1 = _run_sum_layer(pos0[ix_in1], ix_out1, M1, C1, tab1)

    # layer 2: products over h1 (stored unpermuted)
    C2 = _align16(-(-M1 // 128))
    tab2 = _pad_to(h1, 128 * C2).reshape(128, C2)
    h2s, pos2 = _run_prod_layer(
        ix_in2[0::2].reshape(NCORES, -1), ix_in2[1::2].reshape(NCORES, -1),
        C2, tab=tab2)

    # layer 3: segment sums over h2 stream
    C3 = _align16(-(-len(h2s) // 128))
    tab3 = _pad_to(h2s, 128 * C3).reshape(128, C3)
    h3 = _run_sum_layer(pos2[ix_in3], ix_out3, M3, C3, tab3)
    return h3


# revision 21
# speedup vs baseline: 1.1152x; 1.0785x over previous
"""Bass/Trainium2 kernel for nn_CircuitModule (sum-product circuit evaluation).

8 NeuronCores, SPMD, 4 launches (one per circuit layer).  The environment
duty-throttles each engine to short grants per ~29us cycle, and the Pool
(gpsimd) engine effectively retires one instruction per cycle — so the
design minimizes gpsimd instruction count: few, very wide ap_gathers
(W=6144 columns), and batched local_scatter extraction.

  - Product layers (L0, L2): pairs binned by (src-group-of-A, src-group-of-B)
    with coordinated ranks.  Wide ap_gather + fp8 0/1 mask multiply (DVE,
    bf16 out) + block-diagonal ones bf16 matmul collapse the 16 candidate
    partitions per group; PSUM is drained by ACT into bf16 [8, F] streams
    a8d/b8d in DRAM.  A dense pass 2 re-tiles both streams to 128 partitions
    via affine DMA (partition = (s2, a, sp), strip S = K/2), multiplies on
    DVE and stores the f32 product stream.
  - Sum layers (L1, L3): edges binned by (src group, dst block), dst-sorted;
    wide gather + mask + collapse, ACT-drained to bf16 e8d.  Extraction
    re-tiles each group's stream to [128 blocks, L], scans on DVE
    (flag*state + value, fp32 state), local_scatters run-end values (bf16,
    several groups per call), and sums group partials per segment.
  - Host work is index-only preprocessing.
"""

import sys

sys.path.insert(0, "/opt/trn_rl_repo")

import numpy as np

import concourse.bacc as bacc
import concourse.mybir as mybir
import concourse.tile as tile
from concourse import bass_utils

F32 = mybir.dt.float32
BF16 = mybir.dt.bfloat16
F8 = mybir.dt.float8e4
I16 = mybir.dt.int16

MASK_DT = F8
FLAG_DT = BF16

# per-launch HW execution times (ns) from the most recent kernel() call
EXEC_NS = []
TRACE_PATHS = []

NB_VARS = 2_000_000
M0 = 4_000_000
M1 = 1_000_000
M2 = 500_000
M3 = 125_000
NCORES = 8

# x-table geometry: interleaved [pos0, neg0, pos1, neg1, ...], units at 4M
CP = 15_626                 # x_pos entries per partition (padded to 128*CP)
CX = 2 * CP                 # x-table entries per partition (31252 <= 32768)

W = 6144                    # gather chunk width (columns)
PW = 2048                   # PSUM drain piece width (4 banks)


def _pad_to(x, n, val=0):
    out = np.full(n, val, dtype=x.dtype)
    out[: len(x)] = x
    return out


def _align16(n):
    return ((n + 15) // 16) * 16


def _wrap16(idx_groups, F):
    """[8, F] per-group free offsets -> ap_gather wrapped [128, F//16] int16
    (index j of group g is read from partition 16g + j%16, free slot j//16)."""
    assert F % 16 == 0
    assert idx_groups.max(initial=0) < 32768
    out = np.zeros((128, F // 16), dtype=np.int16)
    for g in range(8):
        out[16 * g : 16 * g + 16, :] = (
            idx_groups[g].reshape(F // 16, 16).T.astype(np.int16)
        )
    return out


# ------------------------------------------------------------ host prep


def _prod_maxbin(idxA, idxB, Ctab):
    binid = (idxA // Ctab) // 16 * 8 + (idxB // Ctab) // 16
    return int(np.bincount(binid, minlength=64).max())


def _prep_prod(idxA, idxB, Ctab, K):
    """One core's product-layer prep with forced bin size K (16-aligned)."""
    M = len(idxA)
    F = 8 * K
    a = (idxA // Ctab) // 16
    b = (idxB // Ctab) // 16
    binid = a * 8 + b
    order = np.argsort(binid, kind="stable")
    counts = np.bincount(binid, minlength=64)
    assert counts.max() <= K
    starts = np.zeros(64, dtype=np.int64)
    starts[1:] = np.cumsum(counts)[:-1]
    rank = np.empty(M, dtype=np.int64)
    rank[order] = np.arange(M) - starts[binid[order]]
    colA = b * K + rank                  # column in A row-block a
    colB = a * K + rank                  # column in B row-block b
    store_of_m = b * F + colB            # flat position in the b-layout output
    qA = (idxA % (16 * Ctab)) // Ctab
    qB = (idxB % (16 * Ctab)) // Ctab
    oA = idxA % Ctab
    oB = idxB % Ctab
    idxAg = np.zeros((8, F), dtype=np.int64)
    idxBg = np.zeros((8, F), dtype=np.int64)
    mskA = np.zeros((128, F), dtype=np.float32)
    mskB = np.zeros((128, F), dtype=np.float32)
    idxAg[a, colA] = oA
    idxBg[b, colB] = oB
    mskA[16 * a + qA, colA] = 1.0
    mskB[16 * b + qB, colB] = 1.0
    mdt = mybir.dt.np(MASK_DT)
    return {
        "idxA": _wrap16(idxAg, F),
        "idxB": _wrap16(idxBg, F),
        "mskA": mskA.astype(mdt),
        "mskB": mskB.astype(mdt),
        "store_of_m": store_of_m,
    }


def _balance_blocks(cnt_sg, SB):
    """Greedy 8-dim vector bin-packing: assign S segments (per-group edge
    count vectors cnt_sg [S, 8]) to 128 blocks of capacity SB segments,
    minimizing the max per-(group, block) load.  Returns (blk_of, slot_of,
    maxload)."""
    S = len(cnt_sg)
    order = np.argsort(-cnt_sg.sum(1), kind="stable")
    loads = np.zeros((128, 8), dtype=np.int64)
    nseg = np.zeros(128, dtype=np.int64)
    blk = np.empty(S, dtype=np.int64)
    slot = np.empty(S, dtype=np.int64)
    for s in order:
        cost = np.max(loads + cnt_sg[s], axis=1)
        cost[nseg >= SB] = 1 << 40
        b = int(np.argmin(cost))
        blk[s] = b
        slot[s] = nseg[b]
        nseg[b] += 1
        loads[b] += cnt_sg[s]
    return blk, slot, int(loads.max())


def _prep_sum(idxE, segE, seg_lo, S, Ctab, L, SB, GP, blk_of, slot_of):
    """One core's sum-layer prep.  idxE: src table indices; segE: sorted dst
    segment ids; this core owns S segments starting at seg_lo, assigned to
    128 seg-blocks (<= SB segments each) by the balanced blk_of/slot_of
    maps.  Each (group, block) run of the slot-sorted group stream is
    padded to exactly L positions, so group g's block p occupies stream
    positions [p*L, (p+1)*L) and re-tables to SBUF partition p affinely.
    Run-end partials (bf16) are extracted with local_scatter, GP groups
    per call (scat idx -> g_local*SB + seg slot)."""
    E = len(idxE)
    F = 128 * L
    g = (idxE // Ctab) // 16
    sl = segE - seg_lo
    blk = blk_of[sl]
    slt = slot_of[sl]
    key = g * 128 + blk
    order = np.argsort(key * SB + slt, kind="stable")  # (g, blk, slot)
    gs = g[order]
    bs = blk[order]
    ss = slt[order]
    keys = key[order]
    cnt = np.bincount(key, minlength=1024)
    assert cnt.max() <= L
    starts = np.zeros(1024, dtype=np.int64)
    starts[1:] = np.cumsum(cnt)[:-1]
    rank = np.arange(E) - starts[keys]
    pos = bs * L + rank                      # position in group stream
    q = (idxE % (16 * Ctab)) // Ctab
    o = idxE % Ctab
    idxEg = np.zeros((8, F), dtype=np.int64)
    mskE = np.zeros((128, F), dtype=np.float32)
    idxEg[gs, pos] = o[order]
    mskE[16 * gs + q[order], pos] = 1.0
    # scan flags: 0 at first edge of each (group, slot) run, else 1;
    # laid out [128*8, L]: row 128*g + p = flags of group g's block p
    segg = np.full((8, F), -1, dtype=np.int64)
    segg[gs, pos] = ss
    flags = np.ones((8, F), dtype=np.float32)
    first = np.ones((8, F), dtype=bool)
    first[:, 1:] = segg[:, 1:] != segg[:, :-1]
    # block boundaries always start fresh (segments never span blocks)
    first[:, ::L] = True
    flags[first] = 0.0
    # run-end extraction scatter: GP groups per call; for call c, column
    # (g_local*L + t) -> slot g_local*SB + d  (single bf16 value per slot)
    is_last = np.ones((8, F), dtype=bool)
    is_last[:, :-1] = segg[:, :-1] != segg[:, 1:]
    is_last[:, L - 1 :: L] = True
    gg, jj = np.nonzero(is_last & (segg >= 0))
    dd = segg[gg, jj]
    tt = jj % L
    assert dd.min(initial=0) >= 0 and dd.max(initial=0) < SB
    ncalls = 8 // GP
    scat = np.full((ncalls, 128, GP * L), -1, dtype=np.int16)
    scat[gg // GP, jj // L, (gg % GP) * L + tt] = (
        (gg % GP) * SB + dd
    ).astype(np.int16)
    return {
        "idxE": _wrap16(idxEg, F),
        "mskE": mskE.astype(mybir.dt.np(MASK_DT)),
        "flags": flags.reshape(8, 128, L).reshape(1024, L)
        .astype(mybir.dt.np(FLAG_DT)),
        "scat": scat.reshape(ncalls * 128, GP * L),
    }


# ------------------------------------------------------------ kernels


def _gather_chunk(nc, t_tab, Ctab, idx_dram, msk_dram, c0, w, ip, mp, gp, gbp):
    """Issue idx DMA, wide gather, mask DMA, mask multiply (bf16 out)."""
    t_idx = ip.tile([128, W // 16], I16, tag="idx")
    t_g = gp.tile([128, W], F32, tag="gath")
    t_gb = gbp.tile([128, W], BF16, tag="gathb")
    t_m = mp.tile([128, W], MASK_DT, tag="mask")
    nc.sync.dma_start(out=t_idx[:, : w // 16],
                      in_=idx_dram[:, c0 // 16 : (c0 + w) // 16])
    nc.gpsimd.ap_gather(
        out_ap=t_g[:, :w].rearrange("p (n d) -> p n d", d=1),
        in_ap=t_tab[:].rearrange("p (n d) -> p n d", d=1),
        idxs_ap=t_idx[:, : w // 16],
        channels=128, num_elems=Ctab, d=1, num_idxs=w,
    )
    nc.sync.dma_start(out=t_m[:, :w], in_=msk_dram[:, c0 : c0 + w])
    nc.vector.tensor_tensor(out=t_gb[:, :w], in0=t_g[:, :w], in1=t_m[:, :w],
                            op=mybir.AluOpType.mult)
    return t_gb


def _collapse_store(nc, t_bd, t_gb, w, pp, sp, store):
    """Block-diag ones bf16 matmuls [128 -> 8] in PW pieces; ACT drains each
    piece to a bf16 [8, w] tile and store(tile, piece_lo, piece_w) is called
    to emit the output DMA."""
    for p0 in range(0, w, PW):
        pw = min(PW, w - p0)
        t_ps = pp.tile([8, PW], F32, tag="ps")
        for q0 in range(0, pw, 512):
            qw = min(512, pw - q0)
            nc.tensor.matmul(out=t_ps[:, q0 : q0 + qw], lhsT=t_bd[:],
                             rhs=t_gb[:, p0 + q0 : p0 + q0 + qw],
                             start=True, stop=True)
        t_s8 = sp.tile([8, PW], BF16, tag="s8")
        nc.scalar.copy(out=t_s8[:, :pw], in_=t_ps[:, :pw])
        store(t_s8, p0, pw)


def _build_prod_kernel(Ctab, F, K):
    """Pass 1: collapse streams A and B to bf16 [8, F] DRAM (a8d/b8d).
    Pass 2: out[a, b*K+r] = A[a, b*K+r] * B[b, a*K+r] via affine re-tiling
    to 128 partitions (partition = (s2, a, sp), strip S = K/2)."""
    nc = bacc.Bacc("TRN2")
    S = K // 2
    tabin = nc.dram_tensor("tab", [128, Ctab], F32, kind="ExternalInput")
    idxA = nc.dram_tensor("idxA", [128, F // 16], I16, kind="ExternalInput")
    idxB = nc.dram_tensor("idxB", [128, F // 16], I16, kind="ExternalInput")
    mskA = nc.dram_tensor("mskA", [128, F], MASK_DT, kind="ExternalInput")
    mskB = nc.dram_tensor("mskB", [128, F], MASK_DT, kind="ExternalInput")
    bd = nc.dram_tensor("bd", [128, 8], BF16, kind="ExternalInput")
    out = nc.dram_tensor("out", [8, F], F32, kind="ExternalOutput")
    a8d = nc.dram_tensor("a8d", [8, F], BF16)
    b8d = nc.dram_tensor("b8d", [8, F], BF16)

    with tile.TileContext(nc) as tc:
        with tc.tile_pool(name="const", bufs=1) as cp_:
            t_bd = cp_.tile([128, 8], BF16)
            nc.sync.dma_start(out=t_bd[:], in_=bd[:])
            with (
                tc.tile_pool(name="tabp", bufs=1) as tabp,
                tc.tile_pool(name="idxp", bufs=2) as ip,
                tc.tile_pool(name="mskp", bufs=2) as mp,
                tc.tile_pool(name="gp", bufs=1) as gp,
                tc.tile_pool(name="gbp", bufs=1) as gbp,
                tc.tile_pool(name="s8p", bufs=3) as sp,
                tc.tile_pool(name="ps", bufs=2, space="PSUM") as pp,
            ):
                t_tab = tabp.tile([128, Ctab], F32)
                nc.sync.dma_start(out=t_tab[:], in_=tabin[:])

                def store_b(t_s8, p0, pw, c0):
                    nc.scalar.dma_start(out=b8d[:, c0 + p0 : c0 + p0 + pw],
                                        in_=t_s8[:, :pw])

                def store_a(t_s8, p0, pw, c0):
                    # a8d is in b-layout (flat = b*F + a*K + r); split the
                    # piece's standard col range [y0, y1) at K boundaries
                    y0 = c0 + p0
                    while y0 < c0 + p0 + pw:
                        b = y0 // K
                        yw = min((b + 1) * K, c0 + p0 + pw) - y0
                        r0 = y0 - b * K
                        nc.scalar.dma_start(
                            out=a8d[b].rearrange("(a r) -> a r", r=K)
                            [:, r0 : r0 + yw],
                            in_=t_s8[:, y0 - c0 - p0 : y0 - c0 - p0 + yw])
                        y0 += yw

                # pass 2 (interleaved): both streams are in b-layout
                # (flat = b*F + a*K + r), so the realign multiply for
                # b-block b is a contiguous [16, S] piece (partition
                # p2 = (a, sp), x in [0, S)); block b's inputs complete
                # once phase-A chunks pass column (b+1)*K.
                av = a8d.rearrange("b (p2 x) -> (b p2) x", p2=16)
                bv = b8d.rearrange("b (p2 x) -> (b p2) x", p2=16)
                ov = out.rearrange("b (p2 x) -> (b p2) x", p2=16)
                with (
                    tc.tile_pool(name="pa", bufs=1) as pa,
                    tc.tile_pool(name="pb", bufs=1) as pb,
                    tc.tile_pool(name="po", bufs=1) as po,
                ):

                    def pass2_piece(b):
                        for x0 in range(0, S, PW):
                            xw = min(PW, S - x0)
                            t_a = pa.tile([16, PW], BF16, tag="a")
                            t_b = pb.tile([16, PW], BF16, tag="b")
                            t_o = po.tile([16, PW], F32, tag="o")
                            nc.sync.dma_start(
                                out=t_a[:, :xw],
                                in_=av[16 * b : 16 * (b + 1), x0 : x0 + xw])
                            nc.sync.dma_start(
                                out=t_b[:, :xw],
                                in_=bv[16 * b : 16 * (b + 1), x0 : x0 + xw])
                            nc.vector.tensor_tensor(
                                out=t_o[:, :xw], in0=t_a[:, :xw],
                                in1=t_b[:, :xw], op=mybir.AluOpType.mult)
                            nc.scalar.dma_start(
                                out=ov[16 * b : 16 * (b + 1), x0 : x0 + xw],
                                in_=t_o[:, :xw])

                    for c0 in range(0, F, W):
                        w = min(W, F - c0)
                        t_gb = _gather_chunk(nc, t_tab, Ctab, idxB, mskB,
                                             c0, w, ip, mp, gp, gbp)

                        def storeb(t_s8, p0, pw, c0=c0):
                            store_b(t_s8, p0, pw, c0)

                        _collapse_store(nc, t_bd, t_gb, w, pp, sp, storeb)
                    next_b = 0
                    for c0 in range(0, F, W):
                        w = min(W, F - c0)
                        t_gb = _gather_chunk(nc, t_tab, Ctab, idxA, mskA,
                                             c0, w, ip, mp, gp, gbp)

                        def storea(t_s8, p0, pw, c0=c0):
                            store_a(t_s8, p0, pw, c0)

                        _collapse_store(nc, t_bd, t_gb, w, pp, sp, storea)
                        while next_b < 8 and (next_b + 1) * K <= c0 + w:
                            pass2_piece(next_b)
                            next_b += 1
                    while next_b < 8:
                        pass2_piece(next_b)
                        next_b += 1
    nc.compile()
    return nc


def _build_sum_kernel(Ctab, L, SB, GP):
    nc = bacc.Bacc("TRN2")
    F = 128 * L
    ncalls = 8 // GP
    tabin = nc.dram_tensor("tab", [128, Ctab], F32, kind="ExternalInput")
    idxE = nc.dram_tensor("idxE", [128, F // 16], I16, kind="ExternalInput")
    mskE = nc.dram_tensor("mskE", [128, F], MASK_DT, kind="ExternalInput")
    flags = nc.dram_tensor("flags", [1024, L], FLAG_DT, kind="ExternalInput")
    scat = nc.dram_tensor("scat", [ncalls * 128, GP * L], I16,
                          kind="ExternalInput")
    bd = nc.dram_tensor("bd", [128, 8], BF16, kind="ExternalInput")
    out = nc.dram_tensor("out", [128, SB], F32, kind="ExternalOutput")
    e8d = nc.dram_tensor("e8d", [8, F], BF16)

    with tile.TileContext(nc) as tc:
        with tc.tile_pool(name="const", bufs=1) as cp_:
            t_bd = cp_.tile([128, 8], BF16)
            nc.sync.dma_start(out=t_bd[:], in_=bd[:])

            # stream phase: wide gather, mask, collapse, drain -> e8d (bf16)
            with (
                tc.tile_pool(name="tabp", bufs=1) as tabp,
                tc.tile_pool(name="idxp", bufs=2) as ip,
                tc.tile_pool(name="mskp", bufs=2) as mp,
                tc.tile_pool(name="gp", bufs=1) as gp,
                tc.tile_pool(name="gbp", bufs=1) as gbp,
                tc.tile_pool(name="s8p", bufs=3) as sp,
                tc.tile_pool(name="ps", bufs=2, space="PSUM") as pp,
            ):
                t_tab = tabp.tile([128, Ctab], F32)
                nc.sync.dma_start(out=t_tab[:], in_=tabin[:])
                for c0 in range(0, F, W):
                    w = min(W, F - c0)
                    t_gb = _gather_chunk(nc, t_tab, Ctab, idxE, mskE, c0, w,
                                         ip, mp, gp, gbp)

                    def store(t_s8, p0, pw, c0=c0):
                        nc.scalar.dma_start(
                            out=e8d[:, c0 + p0 : c0 + p0 + pw],
                            in_=t_s8[:, :pw])

                    _collapse_store(nc, t_bd, t_gb, w, pp, sp, store)

            # extraction: per group re-table [128 blocks, L], scan on DVE,
            # batched local_scatter of run-end bf16 values, sum over groups
            with (
                tc.tile_pool(name="xp", bufs=3) as xp,
                tc.tile_pool(name="scp", bufs=1) as scp,
                tc.tile_pool(name="accp", bufs=1) as ap_,
                tc.tile_pool(name="finp", bufs=1) as fin,
            ):
                t_sc8 = scp.tile([128, 8, L], BF16)
                for g in range(8):
                    t_rt = xp.tile([128, L], BF16, tag="rt")
                    nc.sync.dma_start(
                        out=t_rt[:],
                        in_=e8d[g].rearrange("(p l) -> p l", l=L))
                    t_f = xp.tile([128, L], FLAG_DT, tag="fl")
                    nc.sync.dma_start(out=t_f[:],
                                      in_=flags[128 * g : 128 * (g + 1), :])
                    nc.vector.tensor_tensor_scan(
                        out=t_sc8[:, g, :], data0=t_f[:], data1=t_rt[:],
                        initial=0.0,
                        op0=mybir.AluOpType.mult, op1=mybir.AluOpType.add)
                t_acc = ap_.tile([128, ncalls, GP * SB], I16)
                for c in range(ncalls):
                    t_si = xp.tile([128, GP * L], I16, tag="si")
                    nc.sync.dma_start(
                        out=t_si[:],
                        in_=scat[128 * c : 128 * (c + 1), :])
                    nc.gpsimd.local_scatter(
                        out_ap=t_acc[:, c, :],
                        data_ap=t_sc8[:, GP * c : GP * (c + 1), :]
                        .rearrange("p g l -> p (g l)").bitcast(I16),
                        idxs_ap=t_si[:],
                        channels=128, num_elems=GP * SB, num_idxs=GP * L)
                t_accb = t_acc.bitcast(BF16)    # [128, ncalls, GP, SB]
                t_fin = fin.tile([128, SB], F32)
                acv = t_accb.rearrange("p c (g s) -> p (c g) s", g=GP)
                nc.vector.tensor_copy(out=t_fin[:], in_=acv[:, 0, :])
                for j in range(1, 8):
                    nc.vector.tensor_tensor(
                        out=t_fin[:], in0=t_fin[:], in1=acv[:, j, :],
                        op=mybir.AluOpType.add)
                nc.scalar.dma_start(out=out[:], in_=t_fin[:])
    nc.compile()
    return nc


# ------------------------------------------------------------ driver


def _run(nc, in_maps):
    import os

    if os.environ.get("BASS_PROFILE", "0") == "1":
        try:
            import prof_util

            results, ns, tp = prof_util.run_profiled(nc, in_maps, NCORES)
            if ns is not None:
                EXEC_NS.append(ns)
                TRACE_PATHS.append(tp)
            return results
        except ImportError:
            pass
    res = bass_utils.run_bass_kernel_spmd(
        nc, in_maps, list(range(NCORES)), trace=False)
    if res.exec_time_ns is not None:
        EXEC_NS.append(res.exec_time_ns)
    return res.results


def _bd_mat():
    bd = np.zeros((128, 8), dtype=np.float32)
    for g in range(8):
        bd[16 * g : 16 * g + 16, g] = 1.0
    return bd.astype(mybir.dt.np(BF16))


def _balance_swap(idxA, idxB, Ctab):
    """Products are commutative: orient each so unordered (a, b) quad pairs
    split evenly between bins (a, b) and (b, a), halving max-bin excess."""
    a = (idxA // Ctab) // 16
    b = (idxB // Ctab) // 16
    u = np.minimum(a, b)
    v = np.maximum(a, b)
    key = u * 8 + v
    order = np.argsort(key, kind="stable")
    cnt = np.bincount(key, minlength=64)
    starts = np.zeros(64, dtype=np.int64)
    starts[1:] = np.cumsum(cnt)[:-1]
    rank = np.empty(len(a), dtype=np.int64)
    rank[order] = np.arange(len(a)) - starts[key[order]]
    want_ba = (rank % 2 == 1) & (u != v)      # odd ranks take (v, u)
    swap = np.where(want_ba, a != v, a != u)
    outA = np.where(swap, idxB, idxA)
    outB = np.where(swap, idxA, idxB)
    return outA, outB


def _run_prod_layer(idxA_all, idxB_all, Ctab, tab):
    """idxA_all/idxB_all: [NCORES, Mc].  Returns (stored stream, store pos)."""
    bd = _bd_mat()
    sw = [_balance_swap(idxA_all[k], idxB_all[k], Ctab) for k in range(NCORES)]
    idxA_all = np.stack([s[0] for s in sw])
    idxB_all = np.stack([s[1] for s in sw])
    K = _align16(max(_prod_maxbin(idxA_all[k], idxB_all[k], Ctab)
                     for k in range(NCORES)))
    if K % 32:
        K += 16                         # keep S = K/2 16-aligned
    F = 8 * K
    preps = [_prep_prod(idxA_all[k], idxB_all[k], Ctab, K) for k in range(NCORES)]
    nc = _build_prod_kernel(Ctab, F, K)
    in_maps = [
        {"tab": tab, "idxA": p["idxA"], "idxB": p["idxB"], "mskA": p["mskA"],
         "mskB": p["mskB"], "bd": bd}
        for p in preps
    ]
    res = _run(nc, in_maps)
    stream = np.concatenate([res[k]["out"].reshape(-1) for k in range(NCORES)])
    Mc = idxA_all.shape[1]
    pos = np.empty(NCORES * Mc, dtype=np.int64)
    for k in range(NCORES):
        pos[k * Mc : (k + 1) * Mc] = k * 8 * F + preps[k]["store_of_m"]
    return stream, pos


def _run_sum_layer(idxE, segE, nseg, Ctab, tab):
    """Returns the [nseg] segment-sum vector."""
    bd = _bd_mat()
    assert nseg % NCORES == 0
    S = nseg // NCORES
    SB = -(-S // 128)                  # segs per seg-block (partition)
    GP = 8
    while GP > 1 and GP * SB > 2046:
        GP //= 2
    assert GP * SB <= 2046
    seg_splits = [S * k for k in range(NCORES + 1)]
    edge_splits = np.searchsorted(segE, seg_splits)
    assigns = []
    L = 0
    for k in range(NCORES):
        e0, e1 = edge_splits[k], edge_splits[k + 1]
        g = (idxE[e0:e1] // Ctab) // 16
        sl = segE[e0:e1] - seg_splits[k]
        cnt_sg = np.bincount(sl * 8 + g, minlength=S * 8).reshape(S, 8)
        blk_of, slot_of, mx = _balance_blocks(cnt_sg, SB)
        assigns.append((blk_of, slot_of))
        L = max(L, mx)
    L += L % 2          # F = 128*L is 16-aligned for any even L
    preps = []
    for k in range(NCORES):
        e0, e1 = edge_splits[k], edge_splits[k + 1]
        preps.append(_prep_sum(idxE[e0:e1], segE[e0:e1], seg_splits[k], S,
                               Ctab, L, SB, GP, *assigns[k]))
    nc = _build_sum_kernel(Ctab, L, SB, GP)
    in_maps = [
        {"tab": tab, "idxE": p["idxE"], "mskE": p["mskE"], "flags": p["flags"],
         "scat": p["scat"], "bd": bd}
        for p in preps
    ]
    res = _run(nc, in_maps)
    out = np.empty(nseg, dtype=np.float32)
    for k in range(NCORES):
        blk_of, slot_of = assigns[k]
        out[seg_splits[k] : seg_splits[k + 1]] = res[k]["out"][blk_of, slot_of]
    return out


def kernel(x_pos, ix_in0, ix_in1, ix_out1, ix_in2, ix_in3, ix_out3):
    x_pos = np.asarray(x_pos, dtype=np.float32)
    ix_in0 = np.asarray(ix_in0, dtype=np.int64)
    ix_in1 = np.asarray(ix_in1, dtype=np.int64)
    ix_out1 = np.asarray(ix_out1, dtype=np.int64)
    ix_in2 = np.asarray(ix_in2, dtype=np.int64)
    ix_in3 = np.asarray(ix_in3, dtype=np.int64)
    ix_out3 = np.asarray(ix_out3, dtype=np.int64)
    EXEC_NS.clear()
    TRACE_PATHS.clear()

    # layer 0: remap units behind the interleaved vars, gather+multiply.
    # x-table is built host-side: [pos0, neg0, pos1, neg1, ..., 0, 1, pad...]
    ix0 = np.where(ix_in0 >= 2, ix_in0 - 2, 2 * NB_VARS + ix_in0)
    xtab = np.zeros(128 * CX, dtype=np.float32)
    xtab[0 : 2 * NB_VARS : 2] = x_pos
    xtab[1 : 2 * NB_VARS : 2] = 1.0 - x_pos
    xtab[2 * NB_VARS] = 0.0
    xtab[2 * NB_VARS + 1] = 1.0
    h0s, pos0 = _run_prod_layer(
        ix0[0::2].reshape(NCORES, -1), ix0[1::2].reshape(NCORES, -1),
        CX, xtab.reshape(128, CX))

    # layer 1: segment sums over h0 stream
    C1 = _align16(-(-len(h0s) // 128))
    assert C1 <= 32768, f"h0 stream table too wide: {C1}"
    tab1 = _pad_to(h0s, 128 * C1).reshape(128, C1)
    h1 = _run_sum_layer(pos0[ix_in1], ix_out1, M1, C1, tab1)

    # layer 2: products over h1 (stored unpermuted)
    C2 = _align16(-(-M1 // 128))
    tab2 = _pad_to(h1, 128 * C2).reshape(128, C2)
    h2s, pos2 = _run_prod_layer(
        ix_in2[0::2].reshape(NCORES, -1), ix_in2[1::2].reshape(NCORES, -1),
        C2, tab=tab2)

    # layer 3: segment sums over h2 stream
    C3 = _align16(-(-len(h2s) // 128))
    tab3 = _pad_to(h2s, 128 * C3).reshape(128, C3)
    h3 = _run_sum_layer(pos2[ix_in3], ix_out3, M3, C3, tab3)
    return h3


# revision 22
# speedup vs baseline: 1.1161x; 1.0008x over previous
"""Bass/Trainium2 kernel for nn_CircuitModule (sum-product circuit evaluation).

8 NeuronCores, SPMD, 4 launches (one per circuit layer).  The environment
duty-throttles each engine to short grants per ~29us cycle, and the Pool
(gpsimd) engine effectively retires one instruction per cycle — so the
design minimizes gpsimd instruction count: few, very wide ap_gathers
(W=6144 columns), and batched local_scatter extraction.

  - Product layers (L0, L2): pairs binned by (src-group-of-A, src-group-of-B)
    with coordinated ranks.  Wide ap_gather + fp8 0/1 mask multiply (DVE,
    bf16 out) + block-diagonal ones bf16 matmul collapse the 16 candidate
    partitions per group; PSUM is drained by ACT into bf16 [8, F] streams
    a8d/b8d in DRAM.  A dense pass 2 re-tiles both streams to 128 partitions
    via affine DMA (partition = (s2, a, sp), strip S = K/2), multiplies on
    DVE and stores the f32 product stream.
  - Sum layers (L1, L3): edges binned by (src group, dst block), dst-sorted;
    wide gather + mask + collapse, ACT-drained to bf16 e8d.  Extraction
    re-tiles each group's stream to [128 blocks, L], scans on DVE
    (flag*state + value, fp32 state), local_scatters run-end values (bf16,
    several groups per call), and sums group partials per segment.
  - Host work is index-only preprocessing.
"""

import sys

sys.path.insert(0, "/opt/trn_rl_repo")

import numpy as np

import concourse.bacc as bacc
import concourse.mybir as mybir
import concourse.tile as tile
from concourse import bass_utils

F32 = mybir.dt.float32
BF16 = mybir.dt.bfloat16
F8 = mybir.dt.float8e4
I16 = mybir.dt.int16

MASK_DT = F8
FLAG_DT = BF16

# per-launch HW execution times (ns) from the most recent kernel() call
EXEC_NS = []
TRACE_PATHS = []

NB_VARS = 2_000_000
M0 = 4_000_000
M1 = 1_000_000
M2 = 500_000
M3 = 125_000
NCORES = 8

# x-table geometry: interleaved [pos0, neg0, pos1, neg1, ...], units at 4M
CP = 15_626                 # x_pos entries per partition (padded to 128*CP)
CX = 2 * CP                 # x-table entries per partition (31252 <= 32768)

W = 6144                    # gather chunk width (columns)
PW = 2048                   # PSUM drain piece width (4 banks)


def _pad_to(x, n, val=0):
    out = np.full(n, val, dtype=x.dtype)
    out[: len(x)] = x
    return out


def _align16(n):
    return ((n + 15) // 16) * 16


def _wrap16(idx_groups, F):
    """[8, F] per-group free offsets -> ap_gather wrapped [128, F//16] int16
    (index j of group g is read from partition 16g + j%16, free slot j//16)."""
    assert F % 16 == 0
    assert idx_groups.max(initial=0) < 32768
    out = np.zeros((128, F // 16), dtype=np.int16)
    for g in range(8):
        out[16 * g : 16 * g + 16, :] = (
            idx_groups[g].reshape(F // 16, 16).T.astype(np.int16)
        )
    return out


# ------------------------------------------------------------ host prep


def _prod_maxbin(idxA, idxB, Ctab):
    binid = (idxA // Ctab) // 16 * 8 + (idxB // Ctab) // 16
    return int(np.bincount(binid, minlength=64).max())


def _prep_prod(idxA, idxB, Ctab, K):
    """One core's product-layer prep with forced bin size K (16-aligned)."""
    M = len(idxA)
    F = 8 * K
    a = (idxA // Ctab) // 16
    b = (idxB // Ctab) // 16
    binid = a * 8 + b
    order = np.argsort(binid, kind="stable")
    counts = np.bincount(binid, minlength=64)
    assert counts.max() <= K
    starts = np.zeros(64, dtype=np.int64)
    starts[1:] = np.cumsum(counts)[:-1]
    rank = np.empty(M, dtype=np.int64)
    rank[order] = np.arange(M) - starts[binid[order]]
    colA = b * K + rank                  # column in A row-block a
    colB = a * K + rank                  # column in B row-block b
    store_of_m = b * F + colB            # flat position in the b-layout output
    qA = (idxA % (16 * Ctab)) // Ctab
    qB = (idxB % (16 * Ctab)) // Ctab
    oA = idxA % Ctab
    oB = idxB % Ctab
    idxAg = np.zeros((8, F), dtype=np.int64)
    idxBg = np.zeros((8, F), dtype=np.int64)
    mskA = np.zeros((128, F), dtype=np.float32)
    mskB = np.zeros((128, F), dtype=np.float32)
    idxAg[a, colA] = oA
    idxBg[b, colB] = oB
    mskA[16 * a + qA, colA] = 1.0
    mskB[16 * b + qB, colB] = 1.0
    mdt = mybir.dt.np(MASK_DT)
    return {
        "idxA": _wrap16(idxAg, F),
        "idxB": _wrap16(idxBg, F),
        "mskA": mskA.astype(mdt),
        "mskB": mskB.astype(mdt),
        "store_of_m": store_of_m,
    }


def _balance_blocks(cnt_sg, SB):
    """Greedy 8-dim vector bin-packing: assign S segments (per-group edge
    count vectors cnt_sg [S, 8]) to 128 blocks of capacity SB segments,
    minimizing the max per-(group, block) load.  Returns (blk_of, slot_of,
    maxload)."""
    S = len(cnt_sg)
    order = np.argsort(-cnt_sg.sum(1), kind="stable")
    loads = np.zeros((128, 8), dtype=np.int64)
    nseg = np.zeros(128, dtype=np.int64)
    blk = np.empty(S, dtype=np.int64)
    slot = np.empty(S, dtype=np.int64)
    for s in order:
        cost = np.max(loads + cnt_sg[s], axis=1)
        cost[nseg >= SB] = 1 << 40
        b = int(np.argmin(cost))
        blk[s] = b
        slot[s] = nseg[b]
        nseg[b] += 1
        loads[b] += cnt_sg[s]
    return blk, slot, int(loads.max())


def _prep_sum(idxE, segE, seg_lo, S, Ctab, L, SB, GP, blk_of, slot_of):
    """One core's sum-layer prep.  idxE: src table indices; segE: sorted dst
    segment ids; this core owns S segments starting at seg_lo, assigned to
    128 seg-blocks (<= SB segments each) by the balanced blk_of/slot_of
    maps.  Each (group, block) run of the slot-sorted group stream is
    padded to exactly L positions, so group g's block p occupies stream
    positions [p*L, (p+1)*L) and re-tables to SBUF partition p affinely.
    Run-end partials (bf16) are extracted with local_scatter, GP groups
    per call (scat idx -> g_local*SB + seg slot)."""
    E = len(idxE)
    F = 128 * L
    g = (idxE // Ctab) // 16
    sl = segE - seg_lo
    blk = blk_of[sl]
    slt = slot_of[sl]
    key = g * 128 + blk
    order = np.argsort(key * SB + slt, kind="stable")  # (g, blk, slot)
    gs = g[order]
    bs = blk[order]
    ss = slt[order]
    keys = key[order]
    cnt = np.bincount(key, minlength=1024)
    assert cnt.max() <= L
    starts = np.zeros(1024, dtype=np.int64)
    starts[1:] = np.cumsum(cnt)[:-1]
    rank = np.arange(E) - starts[keys]
    pos = bs * L + rank                      # position in group stream
    q = (idxE % (16 * Ctab)) // Ctab
    o = idxE % Ctab
    idxEg = np.zeros((8, F), dtype=np.int64)
    mskE = np.zeros((128, F), dtype=np.float32)
    idxEg[gs, pos] = o[order]
    mskE[16 * gs + q[order], pos] = 1.0
    # scan flags: 0 at first edge of each (group, slot) run, else 1;
    # laid out [128*8, L]: row 128*g + p = flags of group g's block p
    segg = np.full((8, F), -1, dtype=np.int64)
    segg[gs, pos] = ss
    flags = np.ones((8, F), dtype=np.float32)
    first = np.ones((8, F), dtype=bool)
    first[:, 1:] = segg[:, 1:] != segg[:, :-1]
    # block boundaries always start fresh (segments never span blocks)
    first[:, ::L] = True
    flags[first] = 0.0
    # run-end extraction scatter: GP groups per call; for call c, column
    # (g_local*L + t) -> slot g_local*SB + d  (single bf16 value per slot)
    is_last = np.ones((8, F), dtype=bool)
    is_last[:, :-1] = segg[:, :-1] != segg[:, 1:]
    is_last[:, L - 1 :: L] = True
    gg, jj = np.nonzero(is_last & (segg >= 0))
    dd = segg[gg, jj]
    tt = jj % L
    assert dd.min(initial=0) >= 0 and dd.max(initial=0) < SB
    ncalls = 8 // GP
    scat = np.full((ncalls, 128, GP * L), -1, dtype=np.int16)
    scat[gg // GP, jj // L, (gg % GP) * L + tt] = (
        (gg % GP) * SB + dd
    ).astype(np.int16)
    return {
        "idxE": _wrap16(idxEg, F),
        "mskE": mskE.astype(mybir.dt.np(MASK_DT)),
        "flags": flags.reshape(8, 128, L).reshape(1024, L)
        .astype(mybir.dt.np(FLAG_DT)),
        "scat": scat.reshape(ncalls * 128, GP * L),
    }


# ------------------------------------------------------------ kernels


def _gather_chunk(nc, t_tab, Ctab, idx_dram, msk_dram, c0, w, ip, mp, gp, gbp):
    """Issue idx DMA, wide gather, mask DMA, mask multiply (bf16 out)."""
    t_idx = ip.tile([128, W // 16], I16, tag="idx")
    t_g = gp.tile([128, W], F32, tag="gath")
    t_gb = gbp.tile([128, W], BF16, tag="gathb")
    t_m = mp.tile([128, W], MASK_DT, tag="mask")
    nc.sync.dma_start(out=t_idx[:, : w // 16],
                      in_=idx_dram[:, c0 // 16 : (c0 + w) // 16])
    nc.gpsimd.ap_gather(
        out_ap=t_g[:, :w].rearrange("p (n d) -> p n d", d=1),
        in_ap=t_tab[:].rearrange("p (n d) -> p n d", d=1),
        idxs_ap=t_idx[:, : w // 16],
        channels=128, num_elems=Ctab, d=1, num_idxs=w,
    )
    nc.sync.dma_start(out=t_m[:, :w], in_=msk_dram[:, c0 : c0 + w])
    nc.vector.tensor_tensor(out=t_gb[:, :w], in0=t_g[:, :w], in1=t_m[:, :w],
                            op=mybir.AluOpType.mult)
    return t_gb


def _collapse_store(nc, t_bd, t_gb, w, pp, sp, store):
    """Block-diag ones bf16 matmuls [128 -> 8] in PW pieces; ACT drains each
    piece to a bf16 [8, w] tile and store(tile, piece_lo, piece_w) is called
    to emit the output DMA."""
    for p0 in range(0, w, PW):
        pw = min(PW, w - p0)
        t_ps = pp.tile([8, PW], F32, tag="ps")
        for q0 in range(0, pw, 512):
            qw = min(512, pw - q0)
            nc.tensor.matmul(out=t_ps[:, q0 : q0 + qw], lhsT=t_bd[:],
                             rhs=t_gb[:, p0 + q0 : p0 + q0 + qw],
                             start=True, stop=True)
        t_s8 = sp.tile([8, PW], BF16, tag="s8")
        nc.scalar.copy(out=t_s8[:, :pw], in_=t_ps[:, :pw])
        store(t_s8, p0, pw)


def _build_prod_kernel(Ctab, F, K):
    """Pass 1: collapse streams A and B to bf16 [8, F] DRAM (a8d/b8d).
    Pass 2: out[a, b*K+r] = A[a, b*K+r] * B[b, a*K+r] via affine re-tiling
    to 128 partitions (partition = (s2, a, sp), strip S = K/2)."""
    nc = bacc.Bacc("TRN2")
    S = K // 2
    tabin = nc.dram_tensor("tab", [128, Ctab], F32, kind="ExternalInput")
    idxA = nc.dram_tensor("idxA", [128, F // 16], I16, kind="ExternalInput")
    idxB = nc.dram_tensor("idxB", [128, F // 16], I16, kind="ExternalInput")
    mskA = nc.dram_tensor("mskA", [128, F], MASK_DT, kind="ExternalInput")
    mskB = nc.dram_tensor("mskB", [128, F], MASK_DT, kind="ExternalInput")
    bd = nc.dram_tensor("bd", [128, 8], BF16, kind="ExternalInput")
    out = nc.dram_tensor("out", [8, F], F32, kind="ExternalOutput")
    a8d = nc.dram_tensor("a8d", [8, F], BF16)
    b8d = nc.dram_tensor("b8d", [8, F], BF16)

    with tile.TileContext(nc) as tc:
        with tc.tile_pool(name="const", bufs=1) as cp_:
            t_bd = cp_.tile([128, 8], BF16)
            nc.sync.dma_start(out=t_bd[:], in_=bd[:])
            with (
                tc.tile_pool(name="tabp", bufs=1) as tabp,
                tc.tile_pool(name="idxp", bufs=2) as ip,
                tc.tile_pool(name="mskp", bufs=2) as mp,
                tc.tile_pool(name="gp", bufs=1) as gp,
                tc.tile_pool(name="gbp", bufs=1) as gbp,
                tc.tile_pool(name="s8p", bufs=3) as sp,
                tc.tile_pool(name="ps", bufs=2, space="PSUM") as pp,
            ):
                t_tab = tabp.tile([128, Ctab], F32)
                nc.sync.dma_start(out=t_tab[:], in_=tabin[:])

                def store_b(t_s8, p0, pw, c0):
                    nc.scalar.dma_start(out=b8d[:, c0 + p0 : c0 + p0 + pw],
                                        in_=t_s8[:, :pw])

                def store_a(t_s8, p0, pw, c0):
                    # a8d is in b-layout (flat = b*F + a*K + r); split the
                    # piece's standard col range [y0, y1) at K boundaries
                    y0 = c0 + p0
                    while y0 < c0 + p0 + pw:
                        b = y0 // K
                        yw = min((b + 1) * K, c0 + p0 + pw) - y0
                        r0 = y0 - b * K
                        nc.scalar.dma_start(
                            out=a8d[b].rearrange("(a r) -> a r", r=K)
                            [:, r0 : r0 + yw],
                            in_=t_s8[:, y0 - c0 - p0 : y0 - c0 - p0 + yw])
                        y0 += yw

                for dst, idx_d, msk_d in ((store_b, idxB, mskB),
                                          (store_a, idxA, mskA)):
                    for c0 in range(0, F, W):
                        w = min(W, F - c0)
                        t_gb = _gather_chunk(nc, t_tab, Ctab, idx_d, msk_d,
                                             c0, w, ip, mp, gp, gbp)

                        def store(t_s8, p0, pw, c0=c0, dst=dst):
                            dst(t_s8, p0, pw, c0)

                        _collapse_store(nc, t_bd, t_gb, w, pp, sp, store)

            # pass 2: both streams are now in b-layout (flat = b*F + a*K + r)
            # so the realign multiply is a single contiguous [128, S] pass:
            # partition p = (b, a, sp), x in [0, S).
            av = a8d.rearrange("b (p2 x) -> (b p2) x", p2=16)
            bv = b8d.rearrange("b (p2 x) -> (b p2) x", p2=16)
            ov = out.rearrange("b (p2 x) -> (b p2) x", p2=16)
            with (
                tc.tile_pool(name="pa", bufs=1) as pa,
                tc.tile_pool(name="pb", bufs=1) as pb,
                tc.tile_pool(name="po", bufs=1) as po,
            ):
                t_a = pa.tile([128, S], BF16)
                t_b = pb.tile([128, S], BF16)
                t_o = po.tile([128, S], F32)
                nc.sync.dma_start(out=t_a[:], in_=av[:])
                nc.sync.dma_start(out=t_b[:], in_=bv[:])
                nc.vector.tensor_tensor(out=t_o[:], in0=t_a[:], in1=t_b[:],
                                        op=mybir.AluOpType.mult)
                nc.scalar.dma_start(out=ov[:], in_=t_o[:])
    nc.compile()
    return nc


def _build_sum_kernel(Ctab, L, SB, GP):
    nc = bacc.Bacc("TRN2")
    F = 128 * L
    ncalls = 8 // GP
    tabin = nc.dram_tensor("tab", [128, Ctab], F32, kind="ExternalInput")
    idxE = nc.dram_tensor("idxE", [128, F // 16], I16, kind="ExternalInput")
    mskE = nc.dram_tensor("mskE", [128, F], MASK_DT, kind="ExternalInput")
    flags = nc.dram_tensor("flags", [1024, L], FLAG_DT, kind="ExternalInput")
    scat = nc.dram_tensor("scat", [ncalls * 128, GP * L], I16,
                          kind="ExternalInput")
    bd = nc.dram_tensor("bd", [128, 8], BF16, kind="ExternalInput")
    out = nc.dram_tensor("out", [128, SB], F32, kind="ExternalOutput")
    e8d = nc.dram_tensor("e8d", [8, F], BF16)

    with tile.TileContext(nc) as tc:
        with tc.tile_pool(name="const", bufs=1) as cp_:
            t_bd = cp_.tile([128, 8], BF16)
            nc.sync.dma_start(out=t_bd[:], in_=bd[:])

            # stream phase: wide gather, mask, collapse, drain -> e8d (bf16)
            with (
                tc.tile_pool(name="tabp", bufs=1) as tabp,
                tc.tile_pool(name="idxp", bufs=2) as ip,
                tc.tile_pool(name="mskp", bufs=2) as mp,
                tc.tile_pool(name="gp", bufs=1) as gp,
                tc.tile_pool(name="gbp", bufs=1) as gbp,
                tc.tile_pool(name="s8p", bufs=3) as sp,
                tc.tile_pool(name="ps", bufs=2, space="PSUM") as pp,
            ):
                t_tab = tabp.tile([128, Ctab], F32)
                nc.sync.dma_start(out=t_tab[:], in_=tabin[:])
                for c0 in range(0, F, W):
                    w = min(W, F - c0)
                    t_gb = _gather_chunk(nc, t_tab, Ctab, idxE, mskE, c0, w,
                                         ip, mp, gp, gbp)

                    def store(t_s8, p0, pw, c0=c0):
                        nc.scalar.dma_start(
                            out=e8d[:, c0 + p0 : c0 + p0 + pw],
                            in_=t_s8[:, :pw])

                    _collapse_store(nc, t_bd, t_gb, w, pp, sp, store)

            # extraction: per group re-table [128 blocks, L], scan on DVE,
            # batched local_scatter of run-end bf16 values, sum over groups
            with (
                tc.tile_pool(name="xp", bufs=3) as xp,
                tc.tile_pool(name="scp", bufs=1) as scp,
                tc.tile_pool(name="accp", bufs=1) as ap_,
                tc.tile_pool(name="finp", bufs=1) as fin,
            ):
                t_sc8 = scp.tile([128, 8, L], BF16)
                for g in range(8):
                    t_rt = xp.tile([128, L], BF16, tag="rt")
                    nc.sync.dma_start(
                        out=t_rt[:],
                        in_=e8d[g].rearrange("(p l) -> p l", l=L))
                    t_f = xp.tile([128, L], FLAG_DT, tag="fl")
                    nc.sync.dma_start(out=t_f[:],
                                      in_=flags[128 * g : 128 * (g + 1), :])
                    nc.vector.tensor_tensor_scan(
                        out=t_sc8[:, g, :], data0=t_f[:], data1=t_rt[:],
                        initial=0.0,
                        op0=mybir.AluOpType.mult, op1=mybir.AluOpType.add)
                t_acc = ap_.tile([128, ncalls, GP * SB], I16)
                for c in range(ncalls):
                    t_si = xp.tile([128, GP * L], I16, tag="si")
                    nc.sync.dma_start(
                        out=t_si[:],
                        in_=scat[128 * c : 128 * (c + 1), :])
                    nc.gpsimd.local_scatter(
                        out_ap=t_acc[:, c, :],
                        data_ap=t_sc8[:, GP * c : GP * (c + 1), :]
                        .rearrange("p g l -> p (g l)").bitcast(I16),
                        idxs_ap=t_si[:],
                        channels=128, num_elems=GP * SB, num_idxs=GP * L)
                t_accb = t_acc.bitcast(BF16)    # [128, ncalls, GP, SB]
                t_fin = fin.tile([128, SB], F32)
                acv = t_accb.rearrange("p c (g s) -> p (c g) s", g=GP)
                nc.vector.tensor_copy(out=t_fin[:], in_=acv[:, 0, :])
                for j in range(1, 8):
                    nc.vector.tensor_tensor(
                        out=t_fin[:], in0=t_fin[:], in1=acv[:, j, :],
                        op=mybir.AluOpType.add)
                nc.scalar.dma_start(out=out[:], in_=t_fin[:])
    nc.compile()
    return nc


# ------------------------------------------------------------ driver


def _run(nc, in_maps):
    import os

    if os.environ.get("BASS_PROFILE", "0") == "1":
        try:
            import prof_util

            results, ns, tp = prof_util.run_profiled(nc, in_maps, NCORES)
            if ns is not None:
                EXEC_NS.append(ns)
                TRACE_PATHS.append(tp)
            return results
        except ImportError:
            pass
    res = bass_utils.run_bass_kernel_spmd(
        nc, in_maps, list(range(NCORES)), trace=False)
    if res.exec_time_ns is not None:
        EXEC_NS.append(res.exec_time_ns)
    return res.results


def _bd_mat():
    bd = np.zeros((128, 8), dtype=np.float32)
    for g in range(8):
        bd[16 * g : 16 * g + 16, g] = 1.0
    return bd.astype(mybir.dt.np(BF16))


def _balance_swap(idxA, idxB, Ctab):
    """Products are commutative: orient each so unordered (a, b) quad pairs
    split evenly between bins (a, b) and (b, a), halving max-bin excess."""
    a = (idxA // Ctab) // 16
    b = (idxB // Ctab) // 16
    u = np.minimum(a, b)
    v = np.maximum(a, b)
    key = u * 8 + v
    order = np.argsort(key, kind="stable")
    cnt = np.bincount(key, minlength=64)
    starts = np.zeros(64, dtype=np.int64)
    starts[1:] = np.cumsum(cnt)[:-1]
    rank = np.empty(len(a), dtype=np.int64)
    rank[order] = np.arange(len(a)) - starts[key[order]]
    want_ba = (rank % 2 == 1) & (u != v)      # odd ranks take (v, u)
    swap = np.where(want_ba, a != v, a != u)
    outA = np.where(swap, idxB, idxA)
    outB = np.where(swap, idxA, idxB)
    return outA, outB


def _run_prod_layer(idxA_all, idxB_all, Ctab, tab):
    """idxA_all/idxB_all: [NCORES, Mc].  Returns (stored stream, store pos)."""
    bd = _bd_mat()
    sw = [_balance_swap(idxA_all[k], idxB_all[k], Ctab) for k in range(NCORES)]
    idxA_all = np.stack([s[0] for s in sw])
    idxB_all = np.stack([s[1] for s in sw])
    K = _align16(max(_prod_maxbin(idxA_all[k], idxB_all[k], Ctab)
                     for k in range(NCORES)))
    if K % 32:
        K += 16                         # keep S = K/2 16-aligned
    F = 8 * K
    preps = [_prep_prod(idxA_all[k], idxB_all[k], Ctab, K) for k in range(NCORES)]
    nc = _build_prod_kernel(Ctab, F, K)
    in_maps = [
        {"tab": tab, "idxA": p["idxA"], "idxB": p["idxB"], "mskA": p["mskA"],
         "mskB": p["mskB"], "bd": bd}
        for p in preps
    ]
    res = _run(nc, in_maps)
    stream = np.concatenate([res[k]["out"].reshape(-1) for k in range(NCORES)])
    Mc = idxA_all.shape[1]
    pos = np.empty(NCORES * Mc, dtype=np.int64)
    for k in range(NCORES):
        pos[k * Mc : (k + 1) * Mc] = k * 8 * F + preps[k]["store_of_m"]
    return stream, pos


def _run_sum_layer(idxE, segE, nseg, Ctab, tab):
    """Returns the [nseg] segment-sum vector."""
    bd = _bd_mat()
    assert nseg % NCORES == 0
    S = nseg // NCORES
    SB = -(-S // 128)                  # segs per seg-block (partition)
    GP = 8
    while GP > 1 and GP * SB > 2046:
        GP //= 2
    assert GP * SB <= 2046
    seg_splits = [S * k for k in range(NCORES + 1)]
    edge_splits = np.searchsorted(segE, seg_splits)
    assigns = []
    L = 0
    for k in range(NCORES):
        e0, e1 = edge_splits[k], edge_splits[k + 1]
        g = (idxE[e0:e1] // Ctab) // 16
        sl = segE[e0:e1] - seg_splits[k]
        cnt_sg = np.bincount(sl * 8 + g, minlength=S * 8).reshape(S, 8)
        blk_of, slot_of, mx = _balance_blocks(cnt_sg, SB)
        assigns.append((blk_of, slot_of))
        L = max(L, mx)
    L += L % 2          # F = 128*L is 16-aligned for any even L
    preps = []
    for k in range(NCORES):
        e0, e1 = edge_splits[k], edge_splits[k + 1]
        preps.append(_prep_sum(idxE[e0:e1], segE[e0:e1], seg_splits[k], S,
                               Ctab, L, SB, GP, *assigns[k]))
    nc = _build_sum_kernel(Ctab, L, SB, GP)
    in_maps = [
        {"tab": tab, "idxE": p["idxE"], "mskE": p["mskE"], "flags": p["flags"],
         "scat": p["scat"], "bd": bd}
        for p in preps
    ]
    res = _run(nc, in_maps)
    out = np.empty(nseg, dtype=np.float32)
    for k in range(NCORES):
        blk_of, slot_of = assigns[k]
        out[seg_splits[k] : seg_splits[k + 1]] = res[k]["out"][blk_of, slot_of]
    return out


def kernel(x_pos, ix_in0, ix_in1, ix_out1, ix_in2, ix_in3, ix_out3):
    x_pos = np.asarray(x_pos, dtype=np.float32)
    ix_in0 = np.asarray(ix_in0, dtype=np.int64)
    ix_in1 = np.asarray(ix_in1, dtype=np.int64)
    ix_out1 = np.asarray(ix_out1, dtype=np.int64)
    ix_in2 = np.asarray(ix_in2, dtype=np.int64)
    ix_in3 = np.asarray(ix_in3, dtype=np.int64)
    ix_out3 = np.asarray(ix_out3, dtype=np.int64)
    EXEC_NS.clear()
    TRACE_PATHS.clear()

    # layer 0: remap units behind the interleaved vars, gather+multiply.
    # x-table is built host-side: [pos0, neg0, pos1, neg1, ..., 0, 1, pad...]
    ix0 = np.where(ix_in0 >= 2, ix_in0 - 2, 2 * NB_VARS + ix_in0)
    xtab = np.zeros(128 * CX, dtype=np.float32)
    xtab[0 : 2 * NB_VARS : 2] = x_pos
    xtab[1 : 2 * NB_VARS : 2] = 1.0 - x_pos
    xtab[2 * NB_VARS] = 0.0
    xtab[2 * NB_VARS + 1] = 1.0
    h0s, pos0 = _run_prod_layer(
        ix0[0::2].reshape(NCORES, -1), ix0[1::2].reshape(NCORES, -1),
        CX, xtab.reshape(128, CX))

    # layer 1: segment sums over h0 stream
    C1 = _align16(-(-len(h0s) // 128))
    assert C1 <= 32768, f"h0 stream table too wide: {C1}"
    tab1 = _pad_to(h0s, 128 * C1).reshape(128, C1)
    h1 = _run_sum_layer(pos0[ix_in1], ix_out1, M1, C1, tab1)

    # layer 2: products over h1 (stored unpermuted)
    C2 = _align16(-(-M1 // 128))
    tab2 = _pad_to(h1, 128 * C2).reshape(128, C2)
    h2s, pos2 = _run_prod_layer(
        ix_in2[0::2].reshape(NCORES, -1), ix_in2[1::2].reshape(NCORES, -1),
        C2, tab=tab2)

    # layer 3: segment sums over h2 stream
    C3 = _align16(-(-len(h2s) // 128))
    tab3 = _pad_to(h2s, 128 * C3).reshape(128, C3)
    h3 = _run_sum_layer(pos2[ix_in3], ix_out3, M3, C3, tab3)
    return h3
